# revision 1
# baseline (speedup 1.0000x reference)
"""Trainium2 kernel for nn_CustomizedMoGPositionwiseFF (moe_routing).

Strategy (expert-parallel, per the sharding hint):
  - 32 (group, expert) FFN pairs are sharded across 8 NeuronCores (4 each).
  - Routing (group top-2 gate + per-group inner top-2 gate) is computed on
    host at call time; tokens are dispatched (gathered) per expert into the
    per-core shards -- data-dependent sharding, compiled into the NEFF.
  - Each core runs both FFN matmuls + relu for its 4 experts over the tokens
    routed to them, reading each expert weight exactly once (memory regime).
    Weights/activations are shipped as bf16 (f32 PSUM accumulation).
  - Host applies the cheap O(N*D) combine: iw/b2 scaling, scatter-add of the
    two expert contributions per (token, group), per-group post-layernorm,
    group top-2 mixture, and the outer residual.

The kernel output layout on device is u^T = (relu(z W1 + b1) W2)^T per
dispatched token, written as [D/128, 128, CT] so every DMA is dense.
"""

import os
import numpy as np

# Model dims (hardcoded per the contract; match the reference problem)
B, T, D, H = 2, 1024, 512, 2048
G, E, GK, EK = 4, 8, 2, 2
EPS = 1e-5
N = B * T
P = 128
DT = D // P    # 4 d-tiles
HT = H // P    # 16 h-tiles
NCORES = 8
SLOTS = (G * E) // NCORES  # 4 experts per core
CAP_GRAN = 16              # capacity granularity (tokens)
L1_CHUNK = 512             # moving-dim chunk for both layers

_nc_cache = {}
LAST_RESULTS = None       # test harness can inspect (BassKernelResults)


def _ensure_ntff_hook():
    """Register antenv.axon_hooks with the ctypes NTFF profile hook if the
    container's antenv package lacks it (mirrors trn_agent_boot.trn_boot).
    Makes trace=True work; degrades to hook=None when the .so is absent."""
    try:
        from antenv.axon_hooks import get_axon_ntff_profile_hook  # noqa: F401
        return
    except ImportError:
        pass
    import sys
    import types
    import contextlib
    import ctypes

    mod = types.ModuleType("antenv.axon_hooks")
    _state = {"hook": None}

    def set_axon_ntff_profile_hook(h):
        _state["hook"] = h

    def get_axon_ntff_profile_hook():
        return _state["hook"]

    mod.set_axon_ntff_profile_hook = set_axon_ntff_profile_hook
    mod.get_axon_ntff_profile_hook = get_axon_ntff_profile_hook

    so_path = "/opt/axon/libaxon_pjrt.so"
    hook = None
    if os.path.exists(so_path):
        try:
            lib = ctypes.CDLL(so_path)
            if hasattr(lib, "axon_start_nrt_profile"):
                lib.axon_start_nrt_profile.argtypes = [
                    ctypes.POINTER(ctypes.c_int64), ctypes.c_size_t]
                lib.axon_start_nrt_profile.restype = ctypes.c_int64
                lib.axon_stop_nrt_profile.argtypes = [ctypes.c_char_p]
                lib.axon_stop_nrt_profile.restype = ctypes.c_int64

                @contextlib.contextmanager
                def _hook(output_dir, device_ids):
                    import jax
                    jax.devices()
                    if device_ids:
                        ids = (ctypes.c_int64 * len(device_ids))(*device_ids)
                        rc = lib.axon_start_nrt_profile(ids, len(device_ids))
                    else:
                        rc = lib.axon_start_nrt_profile(None, 0)
                    if rc != 0:
                        raise RuntimeError(f"axon_start_nrt_profile rc={rc}")
                    try:
                        yield
                    finally:
                        n = lib.axon_stop_nrt_profile(str(output_dir).encode())
                        print(f"ntff profile: {n} file(s) -> {output_dir}")

                hook = _hook
        except Exception:
            hook = None
    _state["hook"] = hook
    import antenv
    sys.modules["antenv.axon_hooks"] = mod
    antenv.axon_hooks = mod


def _round_up(x, m):
    return ((x + m - 1) // m) * m


def _routing(inp, ln_g, ln_b, wg_group, wg_inner):
    """Replicate the reference gating bit-for-bit on jax-cpu.

    Returns gi [N,GK] group ids, gsc [N,GK] group softmax, z [N,D] f32,
    eis/escs: per-group inner top-k ids/softmax ([N,EK] each).
    """
    import jax
    import jax.numpy as jnp

    cpu = jax.devices("cpu")[0]
    with jax.default_device(cpu):
        x = jnp.asarray(np.asarray(inp, np.float32)).reshape(-1, D)
        gl = x @ jnp.asarray(np.asarray(wg_group, np.float32))
        gv, gi = jax.lax.top_k(gl, GK)
        gsc = jax.nn.softmax(gv, axis=-1)
        m = jnp.mean(x, axis=-1, keepdims=True)
        xc = x - m
        v = jnp.mean(xc * xc, axis=-1, keepdims=True)
        z = xc * jax.lax.rsqrt(v + EPS) * jnp.asarray(np.asarray(ln_g, np.float32)) \
            + jnp.asarray(np.asarray(ln_b, np.float32))
        wgi = jnp.asarray(np.asarray(wg_inner, np.float32))
        eis, escs = [], []
        for g in range(G):
            l = z @ wgi[g]
            ev, ei = jax.lax.top_k(l, EK)
            esc = jax.nn.softmax(ev, axis=-1)
            eis.append(np.asarray(ei))
            escs.append(np.asarray(esc))
    return np.asarray(gi), np.asarray(gsc), np.asarray(z), eis, escs


def _build_nc(Cs, has_b1=False):
    """Build the SPMD Bass program for per-slot capacities Cs (uniform across cores)."""
    import concourse.bass as bass
    import concourse.bacc as bacc
    import concourse.tile as tile
    from concourse import mybir

    f32 = mybir.dt.float32
    bf16 = mybir.dt.bfloat16
    Relu = mybir.ActivationFunctionType.Relu
    add_op = mybir.AluOpType.add
    max_op = mybir.AluOpType.max

    CT = int(sum(Cs))
    offs = np.concatenate([[0], np.cumsum(Cs)]).astype(int)

    nc = bacc.Bacc("TRN2", target_bir_lowering=False)
    # all DRAM layouts are partition-major [128, free] so every DMA is 128
    # contiguous lines (max-size descriptors, cheap HWDGE issue)
    zt_d = nc.declare_dram_parameter("zt", [P, DT * CT], bf16, isOutput=False)
    w1_d = nc.declare_dram_parameter("w1", [SLOTS, P, DT * H], bf16, isOutput=False)
    w2_d = nc.declare_dram_parameter("w2", [SLOTS, P, HT * D], bf16, isOutput=False)
    b1_d = nc.declare_dram_parameter("b1", [P, SLOTS * HT], f32, isOutput=False)
    u_d = nc.declare_dram_parameter("u", [P, DT * CT], bf16, isOutput=True)

    with tile.TileContext(nc) as tc:
        with tc.tile_pool(name="consts", bufs=1) as consts, \
             tc.tile_pool(name="hpool", bufs=2) as hpool, \
             tc.tile_pool(name="hpsum", bufs=2, space="PSUM") as hpsum, \
             tc.tile_pool(name="upsum", bufs=2, space="PSUM") as upsum, \
             tc.tile_pool(name="usb", bufs=3) as usb:

            zt_sb = consts.tile([P, DT * CT], bf16, tag="zt")
            b1_sb = consts.tile([P, SLOTS * HT], f32, tag="b1")
            zero_sb = consts.tile([P, L1_CHUNK], f32, tag="zero")
            nc.vector.memset(zero_sb[:, :], 0.0)
            w1_sb, w2_sb = [], []
            for s in range(SLOTS):
                w1_sb.append(consts.tile([P, DT * H], bf16, tag=f"w1_{s}", name=f"w1s_{s}"))
                w2_sb.append(consts.tile([P, HT * D], bf16, tag=f"w2_{s}", name=f"w2s_{s}"))

            # ---- resident loads.  Slot 0 is dt-interleaved so the first
            # matmuls start after ~800KB; issue alternates between the two
            # HWDGE rings (Sync and Scalar) to overlap descriptor generation.
            for dt in range(DT):
                nc.sync.dma_start(
                    zt_sb[:, dt * CT:(dt + 1) * CT],
                    zt_d[:, dt * CT:(dt + 1) * CT])
                nc.scalar.dma_start(
                    w1_sb[0][:, dt * H:(dt + 1) * H],
                    w1_d[0][:, dt * H:(dt + 1) * H])
            nc.scalar.dma_start(b1_sb[:, :], b1_d[:, :])
            nc.sync.dma_start(w2_sb[0][:, :], w2_d[0][:, :])
            for s in range(1, SLOTS):
                nc.scalar.dma_start(w1_sb[s][:, :], w1_d[s][:, :])
                nc.sync.dma_start(w2_sb[s][:, :], w2_d[s][:, :])

            # ---- compute
            for s in range(SLOTS):
                C = int(Cs[s])
                off = int(offs[s])
                for c0 in range(0, C, L1_CHUNK):
                    W = min(L1_CHUNK, C - c0)
                    h_sb = hpool.tile([P, HT * W], bf16, tag="h")
                    # layer 1: h^T[ht] = relu(W1^T z^T + b1)
                    for ht in range(HT):
                        ph = hpsum.tile([P, W], f32, tag="ph")
                        for dt in range(DT):
                            nc.tensor.matmul(
                                ph[:, :],
                                w1_sb[s][:, dt * H + ht * P: dt * H + (ht + 1) * P],
                                zt_sb[:, dt * CT + off + c0: dt * CT + off + c0 + W],
                                start=(dt == 0),
                                stop=(dt == DT - 1),
                            )
                        if has_b1:
                            # general path: ACT relu with per-partition bias
                            nc.scalar.activation(
                                h_sb[:, ht * W:(ht + 1) * W], ph[:, :], Relu,
                                bias=b1_sb[:, s * HT + ht: s * HT + ht + 1],
                            )
                        else:
                            # DVE TT: relu(x) = max(x, 0) vs a zeros tile
                            # (TensorScalar/ACTIVATE-free path: the TS ISA
                            # struct has too few sync-wait slots for walrus)
                            nc.vector.tensor_max(
                                h_sb[:, ht * W:(ht + 1) * W], ph[:, :],
                                zero_sb[:, :W],
                            )
                    # layer 2: u^T[dt] = sum_ht W2[ht,dt]^T h^T[ht]
                    u_sb = usb.tile([P, DT * W], bf16, tag="u")
                    for dt in range(DT):
                        pu = upsum.tile([P, W], f32, tag="pu")
                        for ht in range(HT):
                            nc.tensor.matmul(
                                pu[:, :],
                                w2_sb[s][:, ht * D + dt * P: ht * D + (dt + 1) * P],
                                h_sb[:, ht * W:(ht + 1) * W],
                                start=(ht == 0),
                                stop=(ht == HT - 1),
                            )
                        nc.vector.tensor_copy(u_sb[:, dt * W:(dt + 1) * W], pu[:, :])
                    # one batched output DMA per slot-chunk on the SWDGE path
                    # (separate queues from the weight-load HWDGE rings)
                    nc.gpsimd.dma_start(
                        u_d.rearrange("p (d c) -> p d c", d=DT)[:, :, off + c0: off + c0 + W],
                        u_sb.rearrange("p (d c) -> p d c", d=DT),
                    )
    nc.compile()
    return nc


def _get_nc(Cs, has_b1):
    key = (tuple(int(c) for c in Cs), bool(has_b1))
    if key not in _nc_cache:
        _nc_cache[key] = _build_nc(key[0], key[1])
    return _nc_cache[key]


def kernel(inp, ln_g, ln_b, wg_group, wg_inner, W1, b1, W2, b2, gln_g, gln_b):
    global LAST_RESULTS
    import jax
    import jax.numpy as jnp
    import ml_dtypes

    inp = np.asarray(inp)
    in_dtype = inp.dtype
    bf = ml_dtypes.bfloat16

    # ---- 1. routing on host (bit-exact replica of the reference gates)
    gi, gsc, z, eis, escs = _routing(inp, ln_g, ln_b, wg_group, wg_inner)
    x = np.asarray(inp, np.float32).reshape(-1, D)

    # token lists per (g, e)
    tok_lists, scale_lists = {}, {}
    for g in range(G):
        in_g = (gi == g).any(axis=1)
        S_g = np.nonzero(in_g)[0]
        ei, esc = eis[g], escs[g]
        for e in range(E):
            sel = ei[S_g] == e           # [|S_g|, EK]
            has = sel.any(axis=1)
            toks = S_g[has]
            w = (esc[S_g] * sel).sum(axis=1)[has]
            tok_lists[(g, e)] = toks
            scale_lists[(g, e)] = w.astype(np.float32)

    # ---- 2. balanced assignment of the 32 pairs to (core, slot)
    pairs = [(g, e) for g in range(G) for e in range(E)]
    pairs.sort(key=lambda p: -len(tok_lists[p]))
    assign = {}           # (core, slot) -> (g, e)
    Cs = []
    for s in range(SLOTS):
        rank = pairs[s * NCORES:(s + 1) * NCORES]
        Cs.append(max(CAP_GRAN, _round_up(max(len(tok_lists[p]) for p in rank), CAP_GRAN)))
        for c, p in enumerate(rank):
            assign[(c, s)] = p
    CT = int(sum(Cs))
    offs = np.concatenate([[0], np.cumsum(Cs)]).astype(int)

    # ---- 3. build per-core input maps
    W1n = np.asarray(W1, np.float32)
    W2n = np.asarray(W2, np.float32)
    b1n = np.asarray(b1, np.float32)
    b2n = np.asarray(b2, np.float32)
    z_bf = z.astype(bf)

    in_maps = []
    for c in range(NCORES):
        # partition-major device layouts (see _build_nc)
        zt_np = np.zeros((P, DT * CT), bf)
        w1_np = np.empty((SLOTS, P, DT * H), bf)
        w2_np = np.empty((SLOTS, P, HT * D), bf)
        b1_np = np.empty((P, SLOTS * HT), np.float32)
        zt_v = zt_np.reshape(P, DT, CT)
        b1_v = b1_np.reshape(P, SLOTS, HT)
        for s in range(SLOTS):
            g, e = assign[(c, s)]
            toks = tok_lists[(g, e)]
            n = len(toks)
            off = offs[s]
            # z^T tile (dt, p, c) -> [p, dt, c]
            zt_v[:, :, off:off + n] = z_bf[toks].T.reshape(DT, P, n).transpose(1, 0, 2)
            w1_np[s] = (
                W1n[g, e].astype(bf).reshape(DT, P, H).transpose(1, 0, 2).reshape(P, DT * H)
            )
            w2_np[s] = (
                W2n[g, e].astype(bf).reshape(HT, P, D).transpose(1, 0, 2).reshape(P, HT * D)
            )
            b1_v[:, s, :] = b1n[g, e].reshape(HT, P).T
        in_maps.append({"zt": zt_np, "w1": w1_np, "w2": w2_np, "b1": b1_np})

    # ---- 4. compile + run on the 8 NeuronCores
    _ensure_ntff_hook()
    from concourse.bass_utils import run_bass_kernel_spmd

    nc = _get_nc(Cs, has_b1=bool(np.any(b1n)))
    res = run_bass_kernel_spmd(
        nc, in_maps, core_ids=list(range(NCORES)),
        trace=bool(int(os.environ.get("KERNEL_TRACE", "0"))),
    )
    LAST_RESULTS = res

    # ---- 5. host combine
    moe = np.zeros((G, N, D), np.float32)
    for c in range(NCORES):
        # u layout [p, dt*CT + c] -> u^T[d, c] -> [CT, D]
        u = (
            np.asarray(res.results[c]["u"], np.float32)
            .reshape(P, DT, CT).transpose(1, 0, 2).reshape(D, CT).T
        )
        for s in range(SLOTS):
            g, e = assign[(c, s)]
            toks = tok_lists[(g, e)]
            n = len(toks)
            w = scale_lists[(g, e)]
            contrib = u[offs[s]:offs[s] + n] * w[:, None] + w[:, None] * b2n[g, e][None, :]
            np.add.at(moe[g], toks, contrib)

    cpu = jax.devices("cpu")[0]
    with jax.default_device(cpu):
        zj = jnp.asarray(z)
        gi_j = jnp.asarray(gi)
        gsc_j = jnp.asarray(gsc)
        gw_dense = jnp.sum(
            jax.nn.one_hot(gi_j, G, dtype=jnp.float32) * gsc_j[..., None], axis=-2
        )  # [N, G]
        out = jnp.zeros((N, D), jnp.float32)
        gg = jnp.asarray(np.asarray(gln_g, np.float32))
        gb = jnp.asarray(np.asarray(gln_b, np.float32))
        for g in range(G):
            t = zj + jnp.asarray(moe[g])
            m = jnp.mean(t, axis=-1, keepdims=True)
            tc_ = t - m
            v = jnp.mean(tc_ * tc_, axis=-1, keepdims=True)
            y = tc_ * jax.lax.rsqrt(v + EPS) * gg[g] + gb[g]
            out = out + gw_dense[:, g:g + 1] * y
        result = np.asarray(out).reshape(B, T, D) + np.asarray(inp, np.float32)

    return result.astype(in_dtype)



# revision 9
# speedup vs baseline: 1.3274x; 1.3274x over previous
"""Trainium2 kernel for nn_CustomizedMoGPositionwiseFF (moe_routing).

Strategy (expert-parallel, per the sharding hint):
  - 32 (group, expert) FFN pairs are sharded across 8 NeuronCores (4 each).
  - Routing (group top-2 gate + per-group inner top-2 gate) is computed on
    host at call time; tokens are dispatched (gathered) per expert into the
    per-core shards -- data-dependent sharding, compiled into the NEFF.
  - Each core runs both FFN matmuls + relu for its 4 experts over the tokens
    routed to them, reading each expert weight exactly once (memory regime).
  - Weights/activations are shipped as fp8 e4m3 (TRN FP8_EXP4, max 240) with
    static scales (z*8, W1*8 -> psum = 64*h; relu+cast to e4m3; W2*8 ->
    psum = 512*u, stored bf16, divided by 512 on host).  Matmuls run in
    DoubleRow perf mode: 256-deep contraction, 2 fp8 MACs/cell/cycle.
  - DMA loads are issued in consumption order in ~0.25MB chunks alternating
    between the two HWDGE rings so the PE starts ~1us in and never starves.
  - Host applies the cheap O(N*D) combine: iw/b2 scaling, scatter-add of the
    two expert contributions per (token, group), per-group post-layernorm,
    group top-2 mixture, and the outer residual.
"""

import os
import numpy as np

# Model dims (hardcoded per the contract; match the reference problem)
B, T, D, H = 2, 1024, 512, 2048
G, E, GK, EK = 4, 8, 2, 2
EPS = 1e-5
N = B * T
P = 128
DT = D // P    # 4 d-tiles
HT = H // P    # 16 h-tiles
NCORES = 8
SLOTS = (G * E) // NCORES  # 4 experts per core
CAP_GRAN = 16              # capacity granularity (tokens)
L1_CHUNK = 512             # moving-dim chunk for both layers (one PSUM bank)

# fp8 static scales: psum1 = SZ*SW1*h ; h stored as e4m3 at scale SH=SZ*SW1;
# psum2 = SH*SW2*u (bf16 out, divided on host).
SZ = 8.0
SW1 = 8.0
SW2 = 8.0
USCALE = SZ * SW1 * SW2    # 512

_nc_cache = {}
LAST_RESULTS = None       # test harness can inspect (BassKernelResults)


def _ensure_ntff_hook():
    """Register antenv.axon_hooks with the ctypes NTFF profile hook if the
    container's antenv package lacks it (mirrors trn_agent_boot.trn_boot).
    Makes trace=True work; degrades to hook=None when the .so is absent."""
    try:
        from antenv.axon_hooks import get_axon_ntff_profile_hook  # noqa: F401
        return
    except ImportError:
        pass
    import sys
    import types
    import contextlib
    import ctypes

    mod = types.ModuleType("antenv.axon_hooks")
    _state = {"hook": None}

    def set_axon_ntff_profile_hook(h):
        _state["hook"] = h

    def get_axon_ntff_profile_hook():
        return _state["hook"]

    mod.set_axon_ntff_profile_hook = set_axon_ntff_profile_hook
    mod.get_axon_ntff_profile_hook = get_axon_ntff_profile_hook

    so_path = "/opt/axon/libaxon_pjrt.so"
    hook = None
    if os.path.exists(so_path):
        try:
            lib = ctypes.CDLL(so_path)
            if hasattr(lib, "axon_start_nrt_profile"):
                lib.axon_start_nrt_profile.argtypes = [
                    ctypes.POINTER(ctypes.c_int64), ctypes.c_size_t]
                lib.axon_start_nrt_profile.restype = ctypes.c_int64
                lib.axon_stop_nrt_profile.argtypes = [ctypes.c_char_p]
                lib.axon_stop_nrt_profile.restype = ctypes.c_int64

                @contextlib.contextmanager
                def _hook(output_dir, device_ids):
                    import jax
                    jax.devices()
                    if device_ids:
                        ids = (ctypes.c_int64 * len(device_ids))(*device_ids)
                        rc = lib.axon_start_nrt_profile(ids, len(device_ids))
                    else:
                        rc = lib.axon_start_nrt_profile(None, 0)
                    if rc != 0:
                        raise RuntimeError(f"axon_start_nrt_profile rc={rc}")
                    try:
                        yield
                    finally:
                        n = lib.axon_stop_nrt_profile(str(output_dir).encode())
                        print(f"ntff profile: {n} file(s) -> {output_dir}")

                hook = _hook
        except Exception:
            hook = None
    _state["hook"] = hook
    import antenv
    sys.modules["antenv.axon_hooks"] = mod
    antenv.axon_hooks = mod


def _round_up(x, m):
    return ((x + m - 1) // m) * m


def _routing(inp, ln_g, ln_b, wg_group, wg_inner):
    """Replicate the reference gating bit-for-bit on jax-cpu.

    Returns gi [N,GK] group ids, gsc [N,GK] group softmax, z [N,D] f32,
    eis/escs: per-group inner top-k ids/softmax ([N,EK] each).
    """
    import jax
    import jax.numpy as jnp

    cpu = jax.devices("cpu")[0]
    with jax.default_device(cpu):
        x = jnp.asarray(np.asarray(inp, np.float32)).reshape(-1, D)
        gl = x @ jnp.asarray(np.asarray(wg_group, np.float32))
        gv, gi = jax.lax.top_k(gl, GK)
        gsc = jax.nn.softmax(gv, axis=-1)
        m = jnp.mean(x, axis=-1, keepdims=True)
        xc = x - m
        v = jnp.mean(xc * xc, axis=-1, keepdims=True)
        z = xc * jax.lax.rsqrt(v + EPS) * jnp.asarray(np.asarray(ln_g, np.float32)) \
            + jnp.asarray(np.asarray(ln_b, np.float32))
        wgi = jnp.asarray(np.asarray(wg_inner, np.float32))
        eis, escs = [], []
        for g in range(G):
            l = z @ wgi[g]
            ev, ei = jax.lax.top_k(l, EK)
            esc = jax.nn.softmax(ev, axis=-1)
            eis.append(np.asarray(ei))
            escs.append(np.asarray(esc))
    return np.asarray(gi), np.asarray(gsc), np.asarray(z), eis, escs


def _build_nc(Cs, has_b1=False):
    """Build the SPMD Bass program for per-slot capacities Cs (uniform across cores)."""
    import concourse.bass as bass
    import concourse.bacc as bacc
    import concourse.tile as tile
    from concourse import mybir

    f32 = mybir.dt.float32
    bf16 = mybir.dt.bfloat16
    fp8 = mybir.dt.float8e4
    DR = mybir.MatmulPerfMode.DoubleRow
    Relu = mybir.ActivationFunctionType.Relu

    CT = int(sum(Cs))
    offs = np.concatenate([[0], np.cumsum(Cs)]).astype(int)

    nc = bacc.Bacc("TRN2", target_bir_lowering=False)
    # all DRAM layouts are partition-major [128, free] so every DMA is 128
    # contiguous lines (max-size descriptors, cheap HWDGE issue)
    zt_d = nc.declare_dram_parameter("zt", [P, DT * CT], fp8, isOutput=False)
    w1_d = nc.declare_dram_parameter("w1", [SLOTS, P, HT * DT * P], fp8, isOutput=False)
    w2_d = nc.declare_dram_parameter("w2", [SLOTS, P, HT * D], fp8, isOutput=False)
    b1_d = nc.declare_dram_parameter("b1", [P, SLOTS * HT], f32, isOutput=False)
    u_d = nc.declare_dram_parameter("u", [P, DT * CT], bf16, isOutput=True)

    W1CH = 4   # w1 load chunks per slot (4 ht-tiles each)
    W2CH = 4   # w2 load chunks per slot (2 j-pairs each)

    with tile.TileContext(nc) as tc:
        with tc.tile_pool(name="consts", bufs=1) as consts, \
             tc.tile_pool(name="hpool", bufs=2) as hpool, \
             tc.tile_pool(name="hpsum", bufs=2, space="PSUM") as hpsum, \
             tc.tile_pool(name="upsum", bufs=1, space="PSUM") as upsum, \
             tc.tile_pool(name="usb", bufs=2) as usb:

            zt_sb = consts.tile([P, DT * CT], fp8, tag="zt")
            b1_sb = consts.tile([P, SLOTS * HT], f32, tag="b1")
            zero_sb = consts.tile([P, L1_CHUNK], f32, tag="zero")
            nc.vector.memset(zero_sb[:, :], 0.0)
            w1_sb, w2_sb = [], []
            for s in range(SLOTS):
                w1_sb.append(consts.tile([P, HT * DT * P], fp8, tag=f"w1_{s}", name=f"w1s_{s}"))
                w2_sb.append(consts.tile([P, HT * D], fp8, tag=f"w2_{s}", name=f"w2s_{s}"))

            # ---- streaming loads, issued in exact consumption order and
            # alternating between the two HWDGE rings (Sync / Scalar).  The
            # PE's first matmul needs only zt[slot0] + the first w1 chunk
            # (~0.4 MB), so compute starts ~1.2us in and the DMA stream then
            # stays ahead of the PE for the rest of the kernel.
            ring = [nc.sync, nc.scalar]
            rix = 0

            def _load(dst, src):
                nonlocal rix
                ring[rix % 2].dma_start(dst, src)
                rix += 1

            if has_b1:
                _load(b1_sb[:, :], b1_d[:, :])
            for s in range(SLOTS):
                C = int(Cs[s])
                off = int(offs[s])
                _load(zt_sb[:, DT * off: DT * (off + C)],
                      zt_d[:, DT * off: DT * (off + C)])
                step1 = (HT // W1CH) * DT * P
                for i in range(W1CH):
                    _load(w1_sb[s][:, i * step1:(i + 1) * step1],
                          w1_d[s][:, i * step1:(i + 1) * step1])
                step2 = (HT // W2CH) * D
                for i in range(W2CH):
                    _load(w2_sb[s][:, i * step2:(i + 1) * step2],
                          w2_d[s][:, i * step2:(i + 1) * step2])

            # ---- compute
            for s in range(SLOTS):
                C = int(Cs[s])
                off = int(offs[s])
                # [P, DT, C] view of this slot's z^T shard
                ztv = zt_sb[:, DT * off: DT * (off + C)].rearrange(
                    "p (dt c) -> p dt c", dt=DT)
                w1v = w1_sb[s].rearrange("p (ht dt c) -> p ht dt c", ht=HT, dt=DT)
                w2v = w2_sb[s].rearrange("p (ht d) -> p ht d", ht=HT)
                eng = 0
                for c0 in range(0, C, L1_CHUNK):
                    W = min(L1_CHUNK, C - c0)
                    h_sb = hpool.tile([P, HT * W], fp8, tag="h")
                    hv = h_sb.rearrange("p (ht c) -> p ht c", ht=HT)
                    # layer 1: psum[ht] = 64*h^T[ht] = (8*W1)^T (8*z^T),
                    # DoubleRow: two 256-deep contraction steps over dt pairs
                    for ht in range(HT):
                        # full 2KB bank: PSUM zero regions are bank-granular,
                        # so concurrent accumulation groups must not share one
                        ph_full = hpsum.tile([P, L1_CHUNK], f32, tag="ph")
                        ph = ph_full[:, :W]
                        for j in range(DT // 2):
                            nc.tensor.matmul(
                                ph[:, :],
                                w1v[:, ht, 2 * j:2 * j + 2, :],
                                ztv[:, 2 * j:2 * j + 2, c0:c0 + W],
                                start=(j == 0),
                                stop=(j == DT // 2 - 1),
                                perf_mode=DR,
                            )
                        # relu + downcast to e4m3 (values <= ~160 < 240 max),
                        # alternating DVE / ACT so neither engine bottlenecks
                        if has_b1:
                            nc.scalar.activation(
                                hv[:, ht, :], ph[:, :], Relu,
                                bias=b1_sb[:, s * HT + ht: s * HT + ht + 1],
                            )
                        elif ht % 2 == 0:
                            nc.vector.tensor_max(hv[:, ht, :], ph[:, :],
                                                 zero_sb[:, :W])
                        else:
                            nc.scalar.activation(hv[:, ht, :], ph[:, :], Relu)
                    # layer 2: psum[dt] = 512*u^T[dt] = (8*W2)^T (64*h^T),
                    # j-pairs outer (matches w2 load order + h production
                    # order), 4 live PSUM banks accumulate the dt tiles
                    u_sb = usb.tile([P, DT * W], bf16, tag="u")
                    pu = [upsum.tile([P, L1_CHUNK], f32, tag=f"pu{dt}", name=f"pu{dt}")[:, :W]
                          for dt in range(DT)]
                    for j in range(HT // 2):
                        for dt in range(DT):
                            nc.tensor.matmul(
                                pu[dt][:, :],
                                w2v[:, 2 * j:2 * j + 2, dt * P:(dt + 1) * P],
                                hv[:, 2 * j:2 * j + 2, :],
                                start=(j == 0),
                                stop=(j == HT // 2 - 1),
                                perf_mode=DR,
                            )
                    for dt in range(DT):
                        if eng % 2 == 0:
                            nc.vector.tensor_copy(u_sb[:, dt * W:(dt + 1) * W],
                                                  pu[dt][:, :])
                        else:
                            nc.scalar.copy(u_sb[:, dt * W:(dt + 1) * W],
                                           pu[dt][:, :])
                        eng += 1
                    # batched output DMA on the SWDGE path (separate queues
                    # from the weight-load HWDGE rings); u_d is slot-major
                    # [p, slot:[dt, C]] so the single-chunk case is contiguous
                    if W == C:
                        nc.gpsimd.dma_start(
                            u_d[:, DT * off: DT * (off + C)], u_sb[:, :])
                    else:
                        nc.gpsimd.dma_start(
                            u_d[:, DT * off: DT * (off + C)].rearrange(
                                "p (d c) -> p d c", d=DT)[:, :, c0:c0 + W],
                            u_sb.rearrange("p (d c) -> p d c", d=DT),
                        )
    nc.compile()
    return nc


def _get_nc(Cs, has_b1):
    key = (tuple(int(c) for c in Cs), bool(has_b1))
    if key not in _nc_cache:
        _nc_cache[key] = _build_nc(key[0], key[1])
    return _nc_cache[key]


def kernel(inp, ln_g, ln_b, wg_group, wg_inner, W1, b1, W2, b2, gln_g, gln_b):
    global LAST_RESULTS
    import jax
    import jax.numpy as jnp
    import ml_dtypes

    inp = np.asarray(inp)
    in_dtype = inp.dtype
    fp8 = ml_dtypes.float8_e4m3  # TRN FP8_EXP4 (max 240), matches dt.float8e4

    # ---- 1. routing on host (bit-exact replica of the reference gates)
    gi, gsc, z, eis, escs = _routing(inp, ln_g, ln_b, wg_group, wg_inner)
    x = np.asarray(inp, np.float32).reshape(-1, D)

    # token lists per (g, e)
    tok_lists, scale_lists = {}, {}
    for g in range(G):
        in_g = (gi == g).any(axis=1)
        S_g = np.nonzero(in_g)[0]
        ei, esc = eis[g], escs[g]
        for e in range(E):
            sel = ei[S_g] == e           # [|S_g|, EK]
            has = sel.any(axis=1)
            toks = S_g[has]
            w = (esc[S_g] * sel).sum(axis=1)[has]
            tok_lists[(g, e)] = toks
            scale_lists[(g, e)] = w.astype(np.float32)

    # ---- 2. balanced assignment of the 32 pairs to (core, slot)
    pairs = [(g, e) for g in range(G) for e in range(E)]
    pairs.sort(key=lambda p: -len(tok_lists[p]))
    assign = {}           # (core, slot) -> (g, e)
    Cs = []
    for s in range(SLOTS):
        rank = pairs[s * NCORES:(s + 1) * NCORES]
        Cs.append(max(CAP_GRAN, _round_up(max(len(tok_lists[p]) for p in rank), CAP_GRAN)))
        for c, p in enumerate(rank):
            assign[(c, s)] = p
    CT = int(sum(Cs))
    offs = np.concatenate([[0], np.cumsum(Cs)]).astype(int)

    # ---- 3. build per-core input maps (fp8, static scales)
    W1n = np.asarray(W1, np.float32)
    W2n = np.asarray(W2, np.float32)
    b1n = np.asarray(b1, np.float32)
    b2n = np.asarray(b2, np.float32)
    z8 = (z * SZ).astype(fp8)

    in_maps = []
    for c in range(NCORES):
        # partition-major device layouts (see _build_nc)
        zt_np = np.zeros((P, DT * CT), fp8)
        w1_np = np.empty((SLOTS, P, HT * DT * P), fp8)
        w2_np = np.empty((SLOTS, P, HT * D), fp8)
        b1_np = np.empty((P, SLOTS * HT), np.float32)
        b1_v = b1_np.reshape(P, SLOTS, HT)
        for s in range(SLOTS):
            g, e = assign[(c, s)]
            toks = tok_lists[(g, e)]
            n = len(toks)
            off = int(offs[s])
            C = int(Cs[s])
            # z^T slot region [p, dt, c]
            reg = zt_np[:, DT * off: DT * (off + C)].reshape(P, DT, C)
            reg[:, :, :n] = z8[toks].T.reshape(DT, P, n).transpose(1, 0, 2)
            # w1 [p, ht, dt, c] = 8*W1[dt*128+p, ht*128+c]
            w1_np[s] = (
                (W1n[g, e] * SW1).astype(fp8)
                .reshape(DT, P, HT, P).transpose(1, 2, 0, 3).reshape(P, HT * DT * P)
            )
            # w2 [p, ht, d] = 8*W2[ht*128+p, d]
            w2_np[s] = (
                (W2n[g, e] * SW2).astype(fp8)
                .reshape(HT, P, D).transpose(1, 0, 2).reshape(P, HT * D)
            )
            # bias lands in psum scale (SZ*SW1)
            b1_v[:, s, :] = (b1n[g, e] * (SZ * SW1)).reshape(HT, P).T
        in_maps.append({"zt": zt_np, "w1": w1_np, "w2": w2_np, "b1": b1_np})

    # ---- 4. compile + run on the 8 NeuronCores
    _ensure_ntff_hook()
    from concourse.bass_utils import run_bass_kernel_spmd

    nc = _get_nc(Cs, has_b1=bool(np.any(b1n)))
    res = run_bass_kernel_spmd(
        nc, in_maps, core_ids=list(range(NCORES)),
        trace=bool(int(os.environ.get("KERNEL_TRACE", "0"))),
    )
    LAST_RESULTS = res

    # ---- 5. host combine
    moe = np.zeros((G, N, D), np.float32)
    for c in range(NCORES):
        uc = np.asarray(res.results[c]["u"], np.float32)
        for s in range(SLOTS):
            g, e = assign[(c, s)]
            toks = tok_lists[(g, e)]
            n = len(toks)
            off = int(offs[s])
            C = int(Cs[s])
            # u slot region [p, dt, c] -> [c, dt*128+p] = 512*u[token, d]
            u = (
                uc[:, DT * off: DT * (off + C)]
                .reshape(P, DT, C).transpose(1, 0, 2).reshape(D, C).T
            )
            w = scale_lists[(g, e)]
            contrib = u[:n] * (w / USCALE)[:, None] + w[:, None] * b2n[g, e][None, :]
            np.add.at(moe[g], toks, contrib)

    cpu = jax.devices("cpu")[0]
    with jax.default_device(cpu):
        zj = jnp.asarray(z)
        gi_j = jnp.asarray(gi)
        gsc_j = jnp.asarray(gsc)
        gw_dense = jnp.sum(
            jax.nn.one_hot(gi_j, G, dtype=jnp.float32) * gsc_j[..., None], axis=-2
        )  # [N, G]
        out = jnp.zeros((N, D), jnp.float32)
        gg = jnp.asarray(np.asarray(gln_g, np.float32))
        gb = jnp.asarray(np.asarray(gln_b, np.float32))
        for g in range(G):
            t = zj + jnp.asarray(moe[g])
            m = jnp.mean(t, axis=-1, keepdims=True)
            tc_ = t - m
            v = jnp.mean(tc_ * tc_, axis=-1, keepdims=True)
            y = tc_ * jax.lax.rsqrt(v + EPS) * gg[g] + gb[g]
            out = out + gw_dense[:, g:g + 1] * y
        result = np.asarray(out).reshape(B, T, D) + np.asarray(inp, np.float32)

    return result.astype(in_dtype)


# revision 11
# speedup vs baseline: 1.4044x; 1.0580x over previous
"""Trainium2 kernel for nn_CustomizedMoGPositionwiseFF (moe_routing).

Strategy (expert-parallel, per the sharding hint):
  - 32 (group, expert) FFN pairs are sharded across 8 NeuronCores (4 each).
  - Routing (group top-2 gate + per-group inner top-2 gate) is computed on
    host at call time; tokens are dispatched (gathered) per expert into the
    per-core shards -- data-dependent sharding, compiled into the NEFF.
  - Each core runs both FFN matmuls + relu for its 4 experts over the tokens
    routed to them, reading each expert weight exactly once (memory regime).
  - Weights/activations are shipped as fp8 e4m3 (TRN FP8_EXP4, max 240) with
    static scales (z*8, W1*8 -> psum = 64*h; relu+cast to e4m3; W2*8 ->
    psum = 512*u, stored bf16, divided by 512 on host).  Matmuls run in
    DoubleRow perf mode: 256-deep contraction, 2 fp8 MACs/cell/cycle.
  - DMA loads are issued in consumption order in ~0.25MB chunks alternating
    between the two HWDGE rings so the PE starts ~1us in and never starves.
  - Host applies the cheap O(N*D) combine: iw/b2 scaling, scatter-add of the
    two expert contributions per (token, group), per-group post-layernorm,
    group top-2 mixture, and the outer residual.
"""

import os
import numpy as np

# Model dims (hardcoded per the contract; match the reference problem)
B, T, D, H = 2, 1024, 512, 2048
G, E, GK, EK = 4, 8, 2, 2
EPS = 1e-5
N = B * T
P = 128
DT = D // P    # 4 d-tiles
HT = H // P    # 16 h-tiles
NCORES = 8
SLOTS = (G * E) // NCORES  # 4 experts per core
CAP_GRAN = 16              # capacity granularity (tokens)
L1_CHUNK = 512             # moving-dim chunk for both layers (one PSUM bank)

# fp8 static scales: psum1 = SZ*SW1*h ; h stored as e4m3 at scale SH=SZ*SW1;
# psum2 = SH*SW2*u (bf16 out, divided on host).
SZ = 8.0
SW1 = 8.0
SW2 = 8.0
USCALE = SZ * SW1 * SW2    # 512

_nc_cache = {}
LAST_RESULTS = None       # test harness can inspect (BassKernelResults)


def _ensure_ntff_hook():
    """Register antenv.axon_hooks with the ctypes NTFF profile hook if the
    container's antenv package lacks it (mirrors trn_agent_boot.trn_boot).
    Makes trace=True work; degrades to hook=None when the .so is absent."""
    try:
        from antenv.axon_hooks import get_axon_ntff_profile_hook  # noqa: F401
        return
    except ImportError:
        pass
    import sys
    import types
    import contextlib
    import ctypes

    mod = types.ModuleType("antenv.axon_hooks")
    _state = {"hook": None}

    def set_axon_ntff_profile_hook(h):
        _state["hook"] = h

    def get_axon_ntff_profile_hook():
        return _state["hook"]

    mod.set_axon_ntff_profile_hook = set_axon_ntff_profile_hook
    mod.get_axon_ntff_profile_hook = get_axon_ntff_profile_hook

    so_path = "/opt/axon/libaxon_pjrt.so"
    hook = None
    if os.path.exists(so_path):
        try:
            lib = ctypes.CDLL(so_path)
            if hasattr(lib, "axon_start_nrt_profile"):
                lib.axon_start_nrt_profile.argtypes = [
                    ctypes.POINTER(ctypes.c_int64), ctypes.c_size_t]
                lib.axon_start_nrt_profile.restype = ctypes.c_int64
                lib.axon_stop_nrt_profile.argtypes = [ctypes.c_char_p]
                lib.axon_stop_nrt_profile.restype = ctypes.c_int64

                @contextlib.contextmanager
                def _hook(output_dir, device_ids):
                    import jax
                    jax.devices()
                    if device_ids:
                        ids = (ctypes.c_int64 * len(device_ids))(*device_ids)
                        rc = lib.axon_start_nrt_profile(ids, len(device_ids))
                    else:
                        rc = lib.axon_start_nrt_profile(None, 0)
                    if rc != 0:
                        raise RuntimeError(f"axon_start_nrt_profile rc={rc}")
                    try:
                        yield
                    finally:
                        n = lib.axon_stop_nrt_profile(str(output_dir).encode())
                        print(f"ntff profile: {n} file(s) -> {output_dir}")

                hook = _hook
        except Exception:
            hook = None
    _state["hook"] = hook
    import antenv
    sys.modules["antenv.axon_hooks"] = mod
    antenv.axon_hooks = mod


def _round_up(x, m):
    return ((x + m - 1) // m) * m


def _routing(inp, ln_g, ln_b, wg_group, wg_inner):
    """Replicate the reference gating bit-for-bit on jax-cpu.

    Returns gi [N,GK] group ids, gsc [N,GK] group softmax, z [N,D] f32,
    eis/escs: per-group inner top-k ids/softmax ([N,EK] each).
    """
    import jax
    import jax.numpy as jnp

    cpu = jax.devices("cpu")[0]
    with jax.default_device(cpu):
        x = jnp.asarray(np.asarray(inp, np.float32)).reshape(-1, D)
        gl = x @ jnp.asarray(np.asarray(wg_group, np.float32))
        gv, gi = jax.lax.top_k(gl, GK)
        gsc = jax.nn.softmax(gv, axis=-1)
        m = jnp.mean(x, axis=-1, keepdims=True)
        xc = x - m
        v = jnp.mean(xc * xc, axis=-1, keepdims=True)
        z = xc * jax.lax.rsqrt(v + EPS) * jnp.asarray(np.asarray(ln_g, np.float32)) \
            + jnp.asarray(np.asarray(ln_b, np.float32))
        wgi = jnp.asarray(np.asarray(wg_inner, np.float32))
        eis, escs = [], []
        for g in range(G):
            l = z @ wgi[g]
            ev, ei = jax.lax.top_k(l, EK)
            esc = jax.nn.softmax(ev, axis=-1)
            eis.append(np.asarray(ei))
            escs.append(np.asarray(esc))
    return np.asarray(gi), np.asarray(gsc), np.asarray(z), eis, escs


def _build_nc(Cs, has_b1=False):
    """Build the SPMD Bass program for per-slot capacities Cs (uniform across cores)."""
    import concourse.bass as bass
    import concourse.bacc as bacc
    import concourse.tile as tile
    from concourse import mybir

    f32 = mybir.dt.float32
    bf16 = mybir.dt.bfloat16
    fp8 = mybir.dt.float8e4
    DR = mybir.MatmulPerfMode.DoubleRow
    Relu = mybir.ActivationFunctionType.Relu

    CT = int(sum(Cs))
    offs = np.concatenate([[0], np.cumsum(Cs)]).astype(int)

    nc = bacc.Bacc("TRN2", target_bir_lowering=False)
    # all DRAM layouts are partition-major [128, free] so every DMA is 128
    # contiguous lines (max-size descriptors, cheap HWDGE issue)
    zt_d = nc.declare_dram_parameter("zt", [P, DT * CT], fp8, isOutput=False)
    w1_d = nc.declare_dram_parameter("w1", [SLOTS, P, HT * DT * P], fp8, isOutput=False)
    w2_d = nc.declare_dram_parameter("w2", [SLOTS, P, HT * D], fp8, isOutput=False)
    b1_d = nc.declare_dram_parameter("b1", [P, SLOTS * HT], f32, isOutput=False)
    u_d = nc.declare_dram_parameter("u", [P, DT * CT], bf16, isOutput=True)

    W1CH = 2   # w1 load chunks per slot
    W2CH = 2   # w2 load chunks per slot

    with tile.TileContext(nc) as tc:
        with tc.tile_pool(name="consts", bufs=1) as consts, \
             tc.tile_pool(name="hpool", bufs=2) as hpool, \
             tc.tile_pool(name="hpsum", bufs=2, space="PSUM") as hpsum, \
             tc.tile_pool(name="upsum", bufs=1, space="PSUM") as upsum, \
             tc.tile_pool(name="usb", bufs=2) as usb:

            zt_sb = consts.tile([P, DT * CT], fp8, tag="zt")
            b1_sb = consts.tile([P, SLOTS * HT], f32, tag="b1")
            zero_sb = consts.tile([P, L1_CHUNK], f32, tag="zero")
            nc.vector.memset(zero_sb[:, :], 0.0)
            w1_sb, w2_sb = [], []
            for s in range(SLOTS):
                w1_sb.append(consts.tile([P, HT * DT * P], fp8, tag=f"w1_{s}", name=f"w1s_{s}"))
                w2_sb.append(consts.tile([P, HT * D], fp8, tag=f"w2_{s}", name=f"w2s_{s}"))

            # ---- streaming loads, issued in exact consumption order, ALL on
            # the Sync engine's HWDGE ring.  Sync runs no compute, so the
            # lane-reuse semaphore waits that pace DMA issue to transfer
            # completion can't block anything (issuing from Scalar serialized
            # the ACT relus behind 25us of paced DMA issues).  One logical
            # queue also makes delivery order == issue order (FIFO per ring).
            def _load(dst, src):
                nc.sync.dma_start(dst, src)

            if has_b1:
                _load(b1_sb[:, :], b1_d[:, :])
            for s in range(SLOTS):
                C = int(Cs[s])
                off = int(offs[s])
                _load(zt_sb[:, DT * off: DT * (off + C)],
                      zt_d[:, DT * off: DT * (off + C)])
                step1 = (HT // W1CH) * DT * P
                for i in range(W1CH):
                    _load(w1_sb[s][:, i * step1:(i + 1) * step1],
                          w1_d[s][:, i * step1:(i + 1) * step1])
                step2 = (HT // W2CH) * D
                for i in range(W2CH):
                    _load(w2_sb[s][:, i * step2:(i + 1) * step2],
                          w2_d[s][:, i * step2:(i + 1) * step2])

            # ---- compute
            for s in range(SLOTS):
                C = int(Cs[s])
                off = int(offs[s])
                # [P, DT, C] view of this slot's z^T shard
                ztv = zt_sb[:, DT * off: DT * (off + C)].rearrange(
                    "p (dt c) -> p dt c", dt=DT)
                w1v = w1_sb[s].rearrange("p (ht dt c) -> p ht dt c", ht=HT, dt=DT)
                w2v = w2_sb[s].rearrange("p (ht d) -> p ht d", ht=HT)
                eng = 0
                for c0 in range(0, C, L1_CHUNK):
                    W = min(L1_CHUNK, C - c0)
                    h_sb = hpool.tile([P, HT * W], fp8, tag="h")
                    hv = h_sb.rearrange("p (ht c) -> p ht c", ht=HT)
                    # layer 1: psum[ht] = 64*h^T[ht] = (8*W1)^T (8*z^T),
                    # DoubleRow: two 256-deep contraction steps over dt pairs
                    for ht in range(HT):
                        # full 2KB bank: PSUM zero regions are bank-granular,
                        # so concurrent accumulation groups must not share one
                        ph_full = hpsum.tile([P, L1_CHUNK], f32, tag="ph")
                        ph = ph_full[:, :W]
                        for j in range(DT // 2):
                            nc.tensor.matmul(
                                ph[:, :],
                                w1v[:, ht, 2 * j:2 * j + 2, :],
                                ztv[:, 2 * j:2 * j + 2, c0:c0 + W],
                                start=(j == 0),
                                stop=(j == DT // 2 - 1),
                                perf_mode=DR,
                            )
                        # relu + downcast to e4m3 (values <= ~160 < 240 max),
                        # alternating DVE / ACT so neither engine bottlenecks
                        if has_b1:
                            nc.scalar.activation(
                                hv[:, ht, :], ph[:, :], Relu,
                                bias=b1_sb[:, s * HT + ht: s * HT + ht + 1],
                            )
                        elif ht % 2 == 0:
                            nc.vector.tensor_max(hv[:, ht, :], ph[:, :],
                                                 zero_sb[:, :W])
                        else:
                            nc.scalar.activation(hv[:, ht, :], ph[:, :], Relu)
                    # layer 2: psum[dt] = 512*u^T[dt] = (8*W2)^T (64*h^T),
                    # j-pairs outer (matches w2 load order + h production
                    # order), 4 live PSUM banks accumulate the dt tiles
                    u_sb = usb.tile([P, DT * W], bf16, tag="u")
                    pu = [upsum.tile([P, L1_CHUNK], f32, tag=f"pu{dt}", name=f"pu{dt}")[:, :W]
                          for dt in range(DT)]
                    for j in range(HT // 2):
                        for dt in range(DT):
                            nc.tensor.matmul(
                                pu[dt][:, :],
                                w2v[:, 2 * j:2 * j + 2, dt * P:(dt + 1) * P],
                                hv[:, 2 * j:2 * j + 2, :],
                                start=(j == 0),
                                stop=(j == HT // 2 - 1),
                                perf_mode=DR,
                            )
                    for dt in range(DT):
                        if eng % 2 == 0:
                            nc.vector.tensor_copy(u_sb[:, dt * W:(dt + 1) * W],
                                                  pu[dt][:, :])
                        else:
                            nc.scalar.copy(u_sb[:, dt * W:(dt + 1) * W],
                                           pu[dt][:, :])
                        eng += 1
                    # batched output DMA on the SWDGE path (separate queues
                    # from the weight-load HWDGE rings); u_d is slot-major
                    # [p, slot:[dt, C]] so the single-chunk case is contiguous
                    if W == C:
                        nc.gpsimd.dma_start(
                            u_d[:, DT * off: DT * (off + C)], u_sb[:, :])
                    else:
                        nc.gpsimd.dma_start(
                            u_d[:, DT * off: DT * (off + C)].rearrange(
                                "p (d c) -> p d c", d=DT)[:, :, c0:c0 + W],
                            u_sb.rearrange("p (d c) -> p d c", d=DT),
                        )
    nc.compile()
    return nc


def _get_nc(Cs, has_b1):
    key = (tuple(int(c) for c in Cs), bool(has_b1))
    if key not in _nc_cache:
        _nc_cache[key] = _build_nc(key[0], key[1])
    return _nc_cache[key]


def kernel(inp, ln_g, ln_b, wg_group, wg_inner, W1, b1, W2, b2, gln_g, gln_b):
    global LAST_RESULTS
    import jax
    import jax.numpy as jnp
    import ml_dtypes

    inp = np.asarray(inp)
    in_dtype = inp.dtype
    fp8 = ml_dtypes.float8_e4m3  # TRN FP8_EXP4 (max 240), matches dt.float8e4

    # ---- 1. routing on host (bit-exact replica of the reference gates)
    gi, gsc, z, eis, escs = _routing(inp, ln_g, ln_b, wg_group, wg_inner)
    x = np.asarray(inp, np.float32).reshape(-1, D)

    # token lists per (g, e)
    tok_lists, scale_lists = {}, {}
    for g in range(G):
        in_g = (gi == g).any(axis=1)
        S_g = np.nonzero(in_g)[0]
        ei, esc = eis[g], escs[g]
        for e in range(E):
            sel = ei[S_g] == e           # [|S_g|, EK]
            has = sel.any(axis=1)
            toks = S_g[has]
            w = (esc[S_g] * sel).sum(axis=1)[has]
            tok_lists[(g, e)] = toks
            scale_lists[(g, e)] = w.astype(np.float32)

    # ---- 2. balanced assignment of the 32 pairs to (core, slot)
    pairs = [(g, e) for g in range(G) for e in range(E)]
    pairs.sort(key=lambda p: -len(tok_lists[p]))
    assign = {}           # (core, slot) -> (g, e)
    Cs = []
    for s in range(SLOTS):
        rank = pairs[s * NCORES:(s + 1) * NCORES]
        Cs.append(max(CAP_GRAN, _round_up(max(len(tok_lists[p]) for p in rank), CAP_GRAN)))
        for c, p in enumerate(rank):
            assign[(c, s)] = p
    CT = int(sum(Cs))
    offs = np.concatenate([[0], np.cumsum(Cs)]).astype(int)

    # ---- 3. build per-core input maps (fp8, static scales)
    W1n = np.asarray(W1, np.float32)
    W2n = np.asarray(W2, np.float32)
    b1n = np.asarray(b1, np.float32)
    b2n = np.asarray(b2, np.float32)
    z8 = (z * SZ).astype(fp8)

    in_maps = []
    for c in range(NCORES):
        # partition-major device layouts (see _build_nc)
        zt_np = np.zeros((P, DT * CT), fp8)
        w1_np = np.empty((SLOTS, P, HT * DT * P), fp8)
        w2_np = np.empty((SLOTS, P, HT * D), fp8)
        b1_np = np.empty((P, SLOTS * HT), np.float32)
        b1_v = b1_np.reshape(P, SLOTS, HT)
        for s in range(SLOTS):
            g, e = assign[(c, s)]
            toks = tok_lists[(g, e)]
            n = len(toks)
            off = int(offs[s])
            C = int(Cs[s])
            # z^T slot region [p, dt, c]
            reg = zt_np[:, DT * off: DT * (off + C)].reshape(P, DT, C)
            reg[:, :, :n] = z8[toks].T.reshape(DT, P, n).transpose(1, 0, 2)
            # w1 [p, ht, dt, c] = 8*W1[dt*128+p, ht*128+c]
            w1_np[s] = (
                (W1n[g, e] * SW1).astype(fp8)
                .reshape(DT, P, HT, P).transpose(1, 2, 0, 3).reshape(P, HT * DT * P)
            )
            # w2 [p, ht, d] = 8*W2[ht*128+p, d]
            w2_np[s] = (
                (W2n[g, e] * SW2).astype(fp8)
                .reshape(HT, P, D).transpose(1, 0, 2).reshape(P, HT * D)
            )
            # bias lands in psum scale (SZ*SW1)
            b1_v[:, s, :] = (b1n[g, e] * (SZ * SW1)).reshape(HT, P).T
        in_maps.append({"zt": zt_np, "w1": w1_np, "w2": w2_np, "b1": b1_np})

    # ---- 4. compile + run on the 8 NeuronCores
    _ensure_ntff_hook()
    from concourse.bass_utils import run_bass_kernel_spmd

    nc = _get_nc(Cs, has_b1=bool(np.any(b1n)))
    res = run_bass_kernel_spmd(
        nc, in_maps, core_ids=list(range(NCORES)),
        trace=bool(int(os.environ.get("KERNEL_TRACE", "0"))),
    )
    LAST_RESULTS = res

    # ---- 5. host combine
    moe = np.zeros((G, N, D), np.float32)
    for c in range(NCORES):
        uc = np.asarray(res.results[c]["u"], np.float32)
        for s in range(SLOTS):
            g, e = assign[(c, s)]
            toks = tok_lists[(g, e)]
            n = len(toks)
            off = int(offs[s])
            C = int(Cs[s])
            # u slot region [p, dt, c] -> [c, dt*128+p] = 512*u[token, d]
            u = (
                uc[:, DT * off: DT * (off + C)]
                .reshape(P, DT, C).transpose(1, 0, 2).reshape(D, C).T
            )
            w = scale_lists[(g, e)]
            contrib = u[:n] * (w / USCALE)[:, None] + w[:, None] * b2n[g, e][None, :]
            np.add.at(moe[g], toks, contrib)

    cpu = jax.devices("cpu")[0]
    with jax.default_device(cpu):
        zj = jnp.asarray(z)
        gi_j = jnp.asarray(gi)
        gsc_j = jnp.asarray(gsc)
        gw_dense = jnp.sum(
            jax.nn.one_hot(gi_j, G, dtype=jnp.float32) * gsc_j[..., None], axis=-2
        )  # [N, G]
        out = jnp.zeros((N, D), jnp.float32)
        gg = jnp.asarray(np.asarray(gln_g, np.float32))
        gb = jnp.asarray(np.asarray(gln_b, np.float32))
        for g in range(G):
            t = zj + jnp.asarray(moe[g])
            m = jnp.mean(t, axis=-1, keepdims=True)
            tc_ = t - m
            v = jnp.mean(tc_ * tc_, axis=-1, keepdims=True)
            y = tc_ * jax.lax.rsqrt(v + EPS) * gg[g] + gb[g]
            out = out + gw_dense[:, g:g + 1] * y
        result = np.asarray(out).reshape(B, T, D) + np.asarray(inp, np.float32)

    return result.astype(in_dtype)


# revision 17
# speedup vs baseline: 1.7097x; 1.2174x over previous
"""Trainium2 kernel for nn_CustomizedMoGPositionwiseFF (moe_routing).

Strategy (expert-parallel, per the sharding hint):
  - 32 (group, expert) FFN pairs are sharded across 8 NeuronCores (4 each).
  - Routing (group top-2 gate + per-group inner top-2 gate) is computed on
    host at call time; tokens are dispatched (gathered) per expert into the
    per-core shards -- data-dependent sharding, compiled into the NEFF.
  - Each core runs both FFN matmuls + relu for its 4 experts over the tokens
    routed to them, reading each expert weight exactly once (memory regime).
  - Weights/activations are shipped as fp8 e4m3 (TRN FP8_EXP4, max 240) with
    static scales (z*8, W1*8 -> psum = 64*h; relu+cast to e4m3; W2*8 ->
    psum = 512*u, stored bf16, divided by 512 on host).  Matmuls run in
    DoubleRow perf mode: 256-deep contraction, 2 fp8 MACs/cell/cycle.
  - DMA loads are issued in consumption order in ~0.25MB chunks alternating
    between the two HWDGE rings so the PE starts ~1us in and never starves.
  - Host applies the cheap O(N*D) combine: iw/b2 scaling, scatter-add of the
    two expert contributions per (token, group), per-group post-layernorm,
    group top-2 mixture, and the outer residual.
"""

import os
import numpy as np

# Model dims (hardcoded per the contract; match the reference problem)
B, T, D, H = 2, 1024, 512, 2048
G, E, GK, EK = 4, 8, 2, 2
EPS = 1e-5
N = B * T
P = 128
DT = D // P    # 4 d-tiles
HT = H // P    # 16 h-tiles
NCORES = 8
SLOTS = (G * E) // NCORES  # 4 experts per core
CAP_GRAN = 16              # capacity granularity (tokens)
L1_CHUNK = 512             # moving-dim chunk for both layers (one PSUM bank)

# fp8 static scales: psum1 = SZ*SW1*h ; h stored as e4m3 at scale SH=SZ*SW1;
# psum2 = SH*SW2*u (bf16 out, divided on host).
SZ = 8.0
SW1 = 8.0
SW2 = 8.0
USCALE = SZ * SW1 * SW2    # 512

_nc_cache = {}
LAST_RESULTS = None       # test harness can inspect (BassKernelResults)


def _ensure_ntff_hook():
    """Register antenv.axon_hooks with the ctypes NTFF profile hook if the
    container's antenv package lacks it (mirrors trn_agent_boot.trn_boot).
    Makes trace=True work; degrades to hook=None when the .so is absent."""
    try:
        from antenv.axon_hooks import get_axon_ntff_profile_hook  # noqa: F401
        return
    except ImportError:
        pass
    import sys
    import types
    import contextlib
    import ctypes

    mod = types.ModuleType("antenv.axon_hooks")
    _state = {"hook": None}

    def set_axon_ntff_profile_hook(h):
        _state["hook"] = h

    def get_axon_ntff_profile_hook():
        return _state["hook"]

    mod.set_axon_ntff_profile_hook = set_axon_ntff_profile_hook
    mod.get_axon_ntff_profile_hook = get_axon_ntff_profile_hook

    so_path = "/opt/axon/libaxon_pjrt.so"
    hook = None
    if os.path.exists(so_path):
        try:
            lib = ctypes.CDLL(so_path)
            if hasattr(lib, "axon_start_nrt_profile"):
                lib.axon_start_nrt_profile.argtypes = [
                    ctypes.POINTER(ctypes.c_int64), ctypes.c_size_t]
                lib.axon_start_nrt_profile.restype = ctypes.c_int64
                lib.axon_stop_nrt_profile.argtypes = [ctypes.c_char_p]
                lib.axon_stop_nrt_profile.restype = ctypes.c_int64

                @contextlib.contextmanager
                def _hook(output_dir, device_ids):
                    import jax
                    jax.devices()
                    if device_ids:
                        ids = (ctypes.c_int64 * len(device_ids))(*device_ids)
                        rc = lib.axon_start_nrt_profile(ids, len(device_ids))
                    else:
                        rc = lib.axon_start_nrt_profile(None, 0)
                    if rc != 0:
                        raise RuntimeError(f"axon_start_nrt_profile rc={rc}")
                    try:
                        yield
                    finally:
                        n = lib.axon_stop_nrt_profile(str(output_dir).encode())
                        print(f"ntff profile: {n} file(s) -> {output_dir}")

                hook = _hook
        except Exception:
            hook = None
    _state["hook"] = hook
    import antenv
    sys.modules["antenv.axon_hooks"] = mod
    antenv.axon_hooks = mod


def _round_up(x, m):
    return ((x + m - 1) // m) * m


def _routing(inp, ln_g, ln_b, wg_group, wg_inner):
    """Replicate the reference gating bit-for-bit on jax-cpu.

    Returns gi [N,GK] group ids, gsc [N,GK] group softmax, z [N,D] f32,
    eis/escs: per-group inner top-k ids/softmax ([N,EK] each).
    """
    import jax
    import jax.numpy as jnp

    cpu = jax.devices("cpu")[0]
    with jax.default_device(cpu):
        x = jnp.asarray(np.asarray(inp, np.float32)).reshape(-1, D)
        gl = x @ jnp.asarray(np.asarray(wg_group, np.float32))
        gv, gi = jax.lax.top_k(gl, GK)
        gsc = jax.nn.softmax(gv, axis=-1)
        m = jnp.mean(x, axis=-1, keepdims=True)
        xc = x - m
        v = jnp.mean(xc * xc, axis=-1, keepdims=True)
        z = xc * jax.lax.rsqrt(v + EPS) * jnp.asarray(np.asarray(ln_g, np.float32)) \
            + jnp.asarray(np.asarray(ln_b, np.float32))
        wgi = jnp.asarray(np.asarray(wg_inner, np.float32))
        eis, escs = [], []
        for g in range(G):
            l = z @ wgi[g]
            ev, ei = jax.lax.top_k(l, EK)
            esc = jax.nn.softmax(ev, axis=-1)
            eis.append(np.asarray(ei))
            escs.append(np.asarray(esc))
    return np.asarray(gi), np.asarray(gsc), np.asarray(z), eis, escs


def _build_nc(Cs, has_b1=False):
    """Build the SPMD Bass program for per-slot capacities Cs (uniform across cores)."""
    import concourse.bass as bass
    import concourse.bacc as bacc
    import concourse.tile as tile
    from concourse import mybir

    f32 = mybir.dt.float32
    bf16 = mybir.dt.bfloat16
    fp8 = mybir.dt.float8e4
    DR = mybir.MatmulPerfMode.DoubleRow
    Relu = mybir.ActivationFunctionType.Relu

    CT = int(sum(Cs))
    offs = np.concatenate([[0], np.cumsum(Cs)]).astype(int)

    nc = bacc.Bacc("TRN2", target_bir_lowering=False)
    # all DRAM layouts are partition-major [128, free] so every DMA is 128
    # contiguous lines (max-size descriptors, cheap HWDGE issue)
    zt_d = nc.declare_dram_parameter("zt", [P, DT * CT], fp8, isOutput=False)
    w1_d = nc.declare_dram_parameter("w1", [SLOTS, P, HT * DT * P], fp8, isOutput=False)
    w2_d = nc.declare_dram_parameter("w2", [SLOTS, P, HT * D], fp8, isOutput=False)
    b1_d = nc.declare_dram_parameter("b1", [P, SLOTS * HT], f32, isOutput=False)
    u_d = nc.declare_dram_parameter("u", [P, DT * CT], bf16, isOutput=True)

    W1CH = 2   # w1 load chunks per slot
    W2CH = 2   # w2 load chunks per slot

    WARMUP = 22  # dummy PE matmuls bridging the preamble -> first-data gap

    with tile.TileContext(nc) as tc:
        with tc.tile_pool(name="consts", bufs=1) as consts, \
             tc.tile_pool(name="hpool", bufs=2) as hpool, \
             tc.tile_pool(name="hpsum", bufs=3, space="PSUM") as hpsum, \
             tc.tile_pool(name="upsum", bufs=1, space="PSUM") as upsum, \
             tc.tile_pool(name="wpsum", bufs=1, space="PSUM") as wpsum, \
             tc.tile_pool(name="usb", bufs=2) as usb:

            zt_sb = consts.tile([P, DT * CT], fp8, tag="zt")
            b1_sb = consts.tile([P, SLOTS * HT], f32, tag="b1")
            zero_sb = consts.tile([P, L1_CHUNK], f32, tag="zero")
            zf8_sb = consts.tile([P, 2 * L1_CHUNK], fp8, tag="zf8")
            nc.vector.memset(zero_sb[:, :], 0.0)
            nc.vector.memset(zf8_sb[:, :], 0.0)
            w1_sb, w2_sb = [], []
            for s in range(SLOTS):
                w1_sb.append(consts.tile([P, HT * DT * P], fp8, tag=f"w1_{s}", name=f"w1s_{s}"))
                w2_sb.append(consts.tile([P, HT * D], fp8, tag=f"w2_{s}", name=f"w2s_{s}"))

            # ---- streaming loads, issued in exact consumption order, ALL on
            # the Sync engine's HWDGE ring.  Sync runs no compute, so the
            # lane-reuse semaphore waits that pace DMA issue to transfer
            # completion can't block anything (issuing from Scalar serialized
            # the ACT relus behind 25us of paced DMA issues).  One logical
            # queue also makes delivery order == issue order (FIFO per ring).
            def _load(dst, src):
                nc.sync.dma_start(dst, src)

            if has_b1:
                _load(b1_sb[:, :], b1_d[:, :])
            for s in range(SLOTS):
                C = int(Cs[s])
                off = int(offs[s])
                _load(zt_sb[:, DT * off: DT * (off + C)],
                      zt_d[:, DT * off: DT * (off + C)])
                step1 = (HT // W1CH) * DT * P
                for i in range(W1CH):
                    _load(w1_sb[s][:, i * step1:(i + 1) * step1],
                          w1_d[s][:, i * step1:(i + 1) * step1])
                step2 = (HT // W2CH) * D
                for i in range(W2CH):
                    _load(w2_sb[s][:, i * step2:(i + 1) * step2],
                          w2_d[s][:, i * step2:(i + 1) * step2])

            # ---- HAM warmup: the tile-framework preamble (barriers + engine
            # table loads) plus first-chunk DMA latency leaves the PE idle
            # for ~10us, which re-throttles its clock to 1.2 GHz for the
            # first ~3.4us of real work.  Bridge the gap with dummy
            # DoubleRow matmuls on an all-zero fp8 tile: they depend only on
            # the memset, keep the PE busy from ~3.5us on, and finish right
            # as the first weight chunk lands.
            zf8v = zf8_sb.rearrange("p (two c) -> p two c", two=2)
            wps = wpsum.tile([P, L1_CHUNK], f32, tag="wps")
            for k in range(WARMUP):
                nc.tensor.matmul(
                    wps[:, :], zf8v[:, :, :P], zf8v[:, :, :],
                    start=True, stop=True, perf_mode=DR)

            # ---- compute
            for s in range(SLOTS):
                C = int(Cs[s])
                off = int(offs[s])
                # [P, DT, C] view of this slot's z^T shard
                ztv = zt_sb[:, DT * off: DT * (off + C)].rearrange(
                    "p (dt c) -> p dt c", dt=DT)
                w1v = w1_sb[s].rearrange("p (ht dt c) -> p ht dt c", ht=HT, dt=DT)
                w2v = w2_sb[s].rearrange("p (ht d) -> p ht d", ht=HT)
                eng = 0
                for c0 in range(0, C, L1_CHUNK):
                    W = min(L1_CHUNK, C - c0)
                    h_sb = hpool.tile([P, HT * W], fp8, tag="h")
                    hv = h_sb.rearrange("p (ht c) -> p ht c", ht=HT)
                    # layer 1: psum[ht] = 64*h^T[ht] = (8*W1)^T (8*z^T),
                    # DoubleRow: two 256-deep contraction steps over dt pairs
                    for ht in range(HT):
                        # full 2KB bank: PSUM zero regions are bank-granular,
                        # so concurrent accumulation groups must not share one
                        ph_full = hpsum.tile([P, L1_CHUNK], f32, tag="ph")
                        ph = ph_full[:, :W]
                        for j in range(DT // 2):
                            nc.tensor.matmul(
                                ph[:, :],
                                w1v[:, ht, 2 * j:2 * j + 2, :],
                                ztv[:, 2 * j:2 * j + 2, c0:c0 + W],
                                start=(j == 0),
                                stop=(j == DT // 2 - 1),
                                perf_mode=DR,
                            )
                        # relu + downcast to e4m3 (values <= ~160 < 240 max),
                        # alternating DVE / ACT so neither engine bottlenecks
                        if has_b1:
                            nc.scalar.activation(
                                hv[:, ht, :], ph[:, :], Relu,
                                bias=b1_sb[:, s * HT + ht: s * HT + ht + 1],
                            )
                        elif ht % 2 == 0:
                            nc.vector.tensor_max(hv[:, ht, :], ph[:, :],
                                                 zero_sb[:, :W])
                        else:
                            nc.scalar.activation(hv[:, ht, :], ph[:, :], Relu)
                    # layer 2: psum[dt] = 512*u^T[dt] = (8*W2)^T (64*h^T),
                    # j-pairs outer (matches w2 load order + h production
                    # order), 4 live PSUM banks accumulate the dt tiles
                    u_sb = usb.tile([P, DT * W], bf16, tag="u")
                    pu = [upsum.tile([P, L1_CHUNK], f32, tag=f"pu{dt}", name=f"pu{dt}")[:, :W]
                          for dt in range(DT)]
                    for j in range(HT // 2):
                        for dt in range(DT):
                            nc.tensor.matmul(
                                pu[dt][:, :],
                                w2v[:, 2 * j:2 * j + 2, dt * P:(dt + 1) * P],
                                hv[:, 2 * j:2 * j + 2, :],
                                start=(j == 0),
                                stop=(j == HT // 2 - 1),
                                perf_mode=DR,
                            )
                    for dt in range(DT):
                        if eng % 2 == 0:
                            nc.vector.tensor_copy(u_sb[:, dt * W:(dt + 1) * W],
                                                  pu[dt][:, :])
                        else:
                            nc.scalar.copy(u_sb[:, dt * W:(dt + 1) * W],
                                           pu[dt][:, :])
                        eng += 1
                    # output store on the same Sync HWDGE ring (it queues
                    # after the input loads; avoids the 2.5us gpsimd SWDGE
                    # drain in the kernel tail); u_d is slot-major
                    # [p, slot:[dt, C]] so the single-chunk case is contiguous
                    if W == C:
                        nc.sync.dma_start(
                            u_d[:, DT * off: DT * (off + C)], u_sb[:, :])
                    else:
                        nc.sync.dma_start(
                            u_d[:, DT * off: DT * (off + C)].rearrange(
                                "p (d c) -> p d c", d=DT)[:, :, c0:c0 + W],
                            u_sb.rearrange("p (d c) -> p d c", d=DT),
                        )
    nc.compile()
    return nc


def _get_nc(Cs, has_b1):
    key = (tuple(int(c) for c in Cs), bool(has_b1))
    if key not in _nc_cache:
        _nc_cache[key] = _build_nc(key[0], key[1])
    return _nc_cache[key]


def kernel(inp, ln_g, ln_b, wg_group, wg_inner, W1, b1, W2, b2, gln_g, gln_b):
    global LAST_RESULTS
    import jax
    import jax.numpy as jnp
    import ml_dtypes

    inp = np.asarray(inp)
    in_dtype = inp.dtype
    fp8 = ml_dtypes.float8_e4m3  # TRN FP8_EXP4 (max 240), matches dt.float8e4

    # ---- 1. routing on host (bit-exact replica of the reference gates)
    gi, gsc, z, eis, escs = _routing(inp, ln_g, ln_b, wg_group, wg_inner)
    x = np.asarray(inp, np.float32).reshape(-1, D)

    # token lists per (g, e)
    tok_lists, scale_lists = {}, {}
    for g in range(G):
        in_g = (gi == g).any(axis=1)
        S_g = np.nonzero(in_g)[0]
        ei, esc = eis[g], escs[g]
        for e in range(E):
            sel = ei[S_g] == e           # [|S_g|, EK]
            has = sel.any(axis=1)
            toks = S_g[has]
            w = (esc[S_g] * sel).sum(axis=1)[has]
            tok_lists[(g, e)] = toks
            scale_lists[(g, e)] = w.astype(np.float32)

    # ---- 2. balanced assignment of the 32 pairs to (core, slot)
    pairs = [(g, e) for g in range(G) for e in range(E)]
    pairs.sort(key=lambda p: -len(tok_lists[p]))
    assign = {}           # (core, slot) -> (g, e)
    Cs = []
    for s in range(SLOTS):
        rank = pairs[s * NCORES:(s + 1) * NCORES]
        Cs.append(max(CAP_GRAN, _round_up(max(len(tok_lists[p]) for p in rank), CAP_GRAN)))
        for c, p in enumerate(rank):
            assign[(c, s)] = p
    CT = int(sum(Cs))
    offs = np.concatenate([[0], np.cumsum(Cs)]).astype(int)

    # ---- 3. build per-core input maps (fp8, static scales)
    W1n = np.asarray(W1, np.float32)
    W2n = np.asarray(W2, np.float32)
    b1n = np.asarray(b1, np.float32)
    b2n = np.asarray(b2, np.float32)
    z8 = (z * SZ).astype(fp8)

    in_maps = []
    for c in range(NCORES):
        # partition-major device layouts (see _build_nc)
        zt_np = np.zeros((P, DT * CT), fp8)
        w1_np = np.empty((SLOTS, P, HT * DT * P), fp8)
        w2_np = np.empty((SLOTS, P, HT * D), fp8)
        b1_np = np.empty((P, SLOTS * HT), np.float32)
        b1_v = b1_np.reshape(P, SLOTS, HT)
        for s in range(SLOTS):
            g, e = assign[(c, s)]
            toks = tok_lists[(g, e)]
            n = len(toks)
            off = int(offs[s])
            C = int(Cs[s])
            # z^T slot region [p, dt, c]
            reg = zt_np[:, DT * off: DT * (off + C)].reshape(P, DT, C)
            reg[:, :, :n] = z8[toks].T.reshape(DT, P, n).transpose(1, 0, 2)
            # w1 [p, ht, dt, c] = 8*W1[dt*128+p, ht*128+c]
            w1_np[s] = (
                (W1n[g, e] * SW1).astype(fp8)
                .reshape(DT, P, HT, P).transpose(1, 2, 0, 3).reshape(P, HT * DT * P)
            )
            # w2 [p, ht, d] = 8*W2[ht*128+p, d]
            w2_np[s] = (
                (W2n[g, e] * SW2).astype(fp8)
                .reshape(HT, P, D).transpose(1, 0, 2).reshape(P, HT * D)
            )
            # bias lands in psum scale (SZ*SW1)
            b1_v[:, s, :] = (b1n[g, e] * (SZ * SW1)).reshape(HT, P).T
        in_maps.append({"zt": zt_np, "w1": w1_np, "w2": w2_np, "b1": b1_np})

    # ---- 4. compile + run on the 8 NeuronCores
    _ensure_ntff_hook()
    from concourse.bass_utils import run_bass_kernel_spmd

    nc = _get_nc(Cs, has_b1=bool(np.any(b1n)))
    res = run_bass_kernel_spmd(
        nc, in_maps, core_ids=list(range(NCORES)),
        trace=bool(int(os.environ.get("KERNEL_TRACE", "0"))),
    )
    LAST_RESULTS = res

    # ---- 5. host combine
    moe = np.zeros((G, N, D), np.float32)
    for c in range(NCORES):
        uc = np.asarray(res.results[c]["u"], np.float32)
        for s in range(SLOTS):
            g, e = assign[(c, s)]
            toks = tok_lists[(g, e)]
            n = len(toks)
            off = int(offs[s])
            C = int(Cs[s])
            # u slot region [p, dt, c] -> [c, dt*128+p] = 512*u[token, d]
            u = (
                uc[:, DT * off: DT * (off + C)]
                .reshape(P, DT, C).transpose(1, 0, 2).reshape(D, C).T
            )
            w = scale_lists[(g, e)]
            contrib = u[:n] * (w / USCALE)[:, None] + w[:, None] * b2n[g, e][None, :]
            np.add.at(moe[g], toks, contrib)

    cpu = jax.devices("cpu")[0]
    with jax.default_device(cpu):
        zj = jnp.asarray(z)
        gi_j = jnp.asarray(gi)
        gsc_j = jnp.asarray(gsc)
        gw_dense = jnp.sum(
            jax.nn.one_hot(gi_j, G, dtype=jnp.float32) * gsc_j[..., None], axis=-2
        )  # [N, G]
        out = jnp.zeros((N, D), jnp.float32)
        gg = jnp.asarray(np.asarray(gln_g, np.float32))
        gb = jnp.asarray(np.asarray(gln_b, np.float32))
        for g in range(G):
            t = zj + jnp.asarray(moe[g])
            m = jnp.mean(t, axis=-1, keepdims=True)
            tc_ = t - m
            v = jnp.mean(tc_ * tc_, axis=-1, keepdims=True)
            y = tc_ * jax.lax.rsqrt(v + EPS) * gg[g] + gb[g]
            out = out + gw_dense[:, g:g + 1] * y
        result = np.asarray(out).reshape(B, T, D) + np.asarray(inp, np.float32)

    return result.astype(in_dtype)


# revision 23
# speedup vs baseline: 1.7753x; 1.0384x over previous
"""Trainium2 kernel for nn_CustomizedMoGPositionwiseFF (moe_routing).

Strategy (expert-parallel, per the sharding hint):
  - 32 (group, expert) FFN pairs are sharded across 8 NeuronCores (4 each).
  - Routing (group top-2 gate + per-group inner top-2 gate) is computed on
    host at call time; tokens are dispatched (gathered) per expert into the
    per-core shards -- data-dependent sharding, compiled into the NEFF.
  - Each core runs both FFN matmuls + relu for its 4 experts over the tokens
    routed to them, reading each expert weight exactly once (memory regime).
  - Weights/activations are shipped as fp8 e4m3 (TRN FP8_EXP4, max 240) with
    static scales (z*8, W1*8 -> psum = 64*h; relu+cast to e4m3; W2*8 ->
    psum = 512*u, stored bf16, divided by 512 on host).  Matmuls run in
    DoubleRow perf mode: 256-deep contraction, 2 fp8 MACs/cell/cycle.
  - DMA loads are issued in consumption order in ~0.25MB chunks alternating
    between the two HWDGE rings so the PE starts ~1us in and never starves.
  - Host applies the cheap O(N*D) combine: iw/b2 scaling, scatter-add of the
    two expert contributions per (token, group), per-group post-layernorm,
    group top-2 mixture, and the outer residual.
"""

import os
import numpy as np

# Model dims (hardcoded per the contract; match the reference problem)
B, T, D, H = 2, 1024, 512, 2048
G, E, GK, EK = 4, 8, 2, 2
EPS = 1e-5
N = B * T
P = 128
DT = D // P    # 4 d-tiles
HT = H // P    # 16 h-tiles
NCORES = 8
SLOTS = (G * E) // NCORES  # 4 experts per core
CAP_GRAN = 16              # capacity granularity (tokens)
L1_CHUNK = 512             # moving-dim chunk for both layers (one PSUM bank)

# fp8 static scales: psum1 = SZ*SW1*h ; h stored as e4m3 at scale SH=SZ*SW1;
# psum2 = SH*SW2*u (bf16 out, divided on host).
SZ = 8.0
SW1 = 8.0
SW2 = 8.0
USCALE = SZ * SW1 * SW2    # 512

_nc_cache = {}
LAST_RESULTS = None       # test harness can inspect (BassKernelResults)


def _ensure_ntff_hook():
    """Register antenv.axon_hooks with the ctypes NTFF profile hook if the
    container's antenv package lacks it (mirrors trn_agent_boot.trn_boot).
    Makes trace=True work; degrades to hook=None when the .so is absent."""
    try:
        from antenv.axon_hooks import get_axon_ntff_profile_hook  # noqa: F401
        return
    except ImportError:
        pass
    import sys
    import types
    import contextlib
    import ctypes

    mod = types.ModuleType("antenv.axon_hooks")
    _state = {"hook": None}

    def set_axon_ntff_profile_hook(h):
        _state["hook"] = h

    def get_axon_ntff_profile_hook():
        return _state["hook"]

    mod.set_axon_ntff_profile_hook = set_axon_ntff_profile_hook
    mod.get_axon_ntff_profile_hook = get_axon_ntff_profile_hook

    so_path = "/opt/axon/libaxon_pjrt.so"
    hook = None
    if os.path.exists(so_path):
        try:
            lib = ctypes.CDLL(so_path)
            if hasattr(lib, "axon_start_nrt_profile"):
                lib.axon_start_nrt_profile.argtypes = [
                    ctypes.POINTER(ctypes.c_int64), ctypes.c_size_t]
                lib.axon_start_nrt_profile.restype = ctypes.c_int64
                lib.axon_stop_nrt_profile.argtypes = [ctypes.c_char_p]
                lib.axon_stop_nrt_profile.restype = ctypes.c_int64

                @contextlib.contextmanager
                def _hook(output_dir, device_ids):
                    import jax
                    jax.devices()
                    if device_ids:
                        ids = (ctypes.c_int64 * len(device_ids))(*device_ids)
                        rc = lib.axon_start_nrt_profile(ids, len(device_ids))
                    else:
                        rc = lib.axon_start_nrt_profile(None, 0)
                    if rc != 0:
                        raise RuntimeError(f"axon_start_nrt_profile rc={rc}")
                    try:
                        yield
                    finally:
                        n = lib.axon_stop_nrt_profile(str(output_dir).encode())
                        print(f"ntff profile: {n} file(s) -> {output_dir}")

                hook = _hook
        except Exception:
            hook = None
    _state["hook"] = hook
    import antenv
    sys.modules["antenv.axon_hooks"] = mod
    antenv.axon_hooks = mod


def _round_up(x, m):
    return ((x + m - 1) // m) * m


def _routing(inp, ln_g, ln_b, wg_group, wg_inner):
    """Replicate the reference gating bit-for-bit on jax-cpu.

    Returns gi [N,GK] group ids, gsc [N,GK] group softmax, z [N,D] f32,
    eis/escs: per-group inner top-k ids/softmax ([N,EK] each).
    """
    import jax
    import jax.numpy as jnp

    cpu = jax.devices("cpu")[0]
    with jax.default_device(cpu):
        x = jnp.asarray(np.asarray(inp, np.float32)).reshape(-1, D)
        gl = x @ jnp.asarray(np.asarray(wg_group, np.float32))
        gv, gi = jax.lax.top_k(gl, GK)
        gsc = jax.nn.softmax(gv, axis=-1)
        m = jnp.mean(x, axis=-1, keepdims=True)
        xc = x - m
        v = jnp.mean(xc * xc, axis=-1, keepdims=True)
        z = xc * jax.lax.rsqrt(v + EPS) * jnp.asarray(np.asarray(ln_g, np.float32)) \
            + jnp.asarray(np.asarray(ln_b, np.float32))
        wgi = jnp.asarray(np.asarray(wg_inner, np.float32))
        eis, escs = [], []
        for g in range(G):
            l = z @ wgi[g]
            ev, ei = jax.lax.top_k(l, EK)
            esc = jax.nn.softmax(ev, axis=-1)
            eis.append(np.asarray(ei))
            escs.append(np.asarray(esc))
    return np.asarray(gi), np.asarray(gsc), np.asarray(z), eis, escs


def _build_nc(Cs, has_b1=False):
    """Build the SPMD Bass program for per-slot capacities Cs (uniform across cores)."""
    import concourse.bass as bass
    import concourse.bacc as bacc
    import concourse.tile as tile
    from concourse import mybir

    f32 = mybir.dt.float32
    bf16 = mybir.dt.bfloat16
    fp8 = mybir.dt.float8e4
    DR = mybir.MatmulPerfMode.DoubleRow
    Relu = mybir.ActivationFunctionType.Relu

    CT = int(sum(Cs))
    offs = np.concatenate([[0], np.cumsum(Cs)]).astype(int)

    nc = bacc.Bacc("TRN2", target_bir_lowering=False)
    # all DRAM layouts are partition-major [128, free] so every DMA is 128
    # contiguous lines (max-size descriptors, cheap HWDGE issue)
    zt_d = nc.declare_dram_parameter("zt", [P, DT * CT], fp8, isOutput=False)
    w1_d = nc.declare_dram_parameter("w1", [SLOTS, P, HT * DT * P], fp8, isOutput=False)
    w2_d = nc.declare_dram_parameter("w2", [SLOTS, P, HT * D], fp8, isOutput=False)
    b1_d = nc.declare_dram_parameter("b1", [P, SLOTS * HT], f32, isOutput=False)
    u_d = nc.declare_dram_parameter("u", [P, DT * CT], bf16, isOutput=True)

    W1CH = 2   # w1 load chunks per slot
    W2CH = 2   # w2 load chunks per slot

    WARMUP = 20  # dummy PE matmuls bridging the preamble -> first-data gap

    with tile.TileContext(nc) as tc:
        with tc.tile_pool(name="consts", bufs=1) as consts, \
             tc.tile_pool(name="hpool", bufs=2) as hpool, \
             tc.tile_pool(name="hpsum", bufs=3, space="PSUM") as hpsum, \
             tc.tile_pool(name="upsum", bufs=1, space="PSUM") as upsum, \
             tc.tile_pool(name="wpsum", bufs=1, space="PSUM") as wpsum, \
             tc.tile_pool(name="usb", bufs=2) as usb:

            zt_sb = consts.tile([P, DT * CT], fp8, tag="zt")
            b1_sb = consts.tile([P, SLOTS * HT], f32, tag="b1")
            zero_sb = consts.tile([P, L1_CHUNK], f32, tag="zero")
            zf8_sb = consts.tile([P, L1_CHUNK], fp8, tag="zf8")
            # zf8 first: it gates the HAM-warmup matmuls
            nc.vector.memset(zf8_sb[:, :], 0.0)
            nc.vector.memset(zero_sb[:, :], 0.0)
            w1_sb, w2_sb = [], []
            for s in range(SLOTS):
                w1_sb.append(consts.tile([P, HT * DT * P], fp8, tag=f"w1_{s}", name=f"w1s_{s}"))
                w2_sb.append(consts.tile([P, HT * D], fp8, tag=f"w2_{s}", name=f"w2s_{s}"))

            # ---- streaming loads, issued in exact consumption order, ALL on
            # the Sync engine's HWDGE ring.  Sync runs no compute, so the
            # lane-reuse semaphore waits that pace DMA issue to transfer
            # completion can't block anything (issuing from Scalar serialized
            # the ACT relus behind 25us of paced DMA issues).  One logical
            # queue also makes delivery order == issue order (FIFO per ring).
            def _load(dst, src):
                nc.sync.dma_start(dst, src)

            if has_b1:
                _load(b1_sb[:, :], b1_d[:, :])
            for s in range(SLOTS):
                C = int(Cs[s])
                off = int(offs[s])
                _load(zt_sb[:, DT * off: DT * (off + C)],
                      zt_d[:, DT * off: DT * (off + C)])
                # slot 0's first w1 chunk is small so the first real matmul
                # can start as early as possible
                hts = [2, 6, 8] if s == 0 else [8, 8]
                h0 = 0
                for nh in hts:
                    _load(w1_sb[s][:, h0 * DT * P:(h0 + nh) * DT * P],
                          w1_d[s][:, h0 * DT * P:(h0 + nh) * DT * P])
                    h0 += nh
                step2 = (HT // W2CH) * D
                for i in range(W2CH):
                    _load(w2_sb[s][:, i * step2:(i + 1) * step2],
                          w2_d[s][:, i * step2:(i + 1) * step2])

            # ---- HAM warmup: the tile-framework preamble (barriers + engine
            # table loads) plus first-chunk DMA latency leaves the PE idle
            # for ~10us, which re-throttles its clock to 1.2 GHz for the
            # first ~3.4us of real work.  Bridge the gap with dummy
            # DoubleRow matmuls on an all-zero fp8 tile: they depend only on
            # the memset, keep the PE busy from ~3.5us on, and finish right
            # as the first weight chunk lands.
            zf8v = zf8_sb.rearrange("p (two c) -> p two c", two=2)
            wps = wpsum.tile([P, L1_CHUNK], f32, tag="wps")
            for k in range(WARMUP):
                nc.tensor.matmul(
                    wps[:, :L1_CHUNK // 2], zf8v[:, :, :P], zf8v[:, :, :],
                    start=True, stop=True, perf_mode=DR)

            # ---- compute
            for s in range(SLOTS):
                C = int(Cs[s])
                off = int(offs[s])
                # [P, DT, C] view of this slot's z^T shard
                ztv = zt_sb[:, DT * off: DT * (off + C)].rearrange(
                    "p (dt c) -> p dt c", dt=DT)
                w1v = w1_sb[s].rearrange("p (ht dt c) -> p ht dt c", ht=HT, dt=DT)
                w2v = w2_sb[s].rearrange("p (ht d) -> p ht d", ht=HT)
                eng = 0
                for c0 in range(0, C, L1_CHUNK):
                    W = min(L1_CHUNK, C - c0)
                    h_sb = hpool.tile([P, HT * W], fp8, tag="h")
                    hv = h_sb.rearrange("p (ht c) -> p ht c", ht=HT)
                    # layer 1: psum[ht] = 64*h^T[ht] = (8*W1)^T (8*z^T),
                    # DoubleRow: two 256-deep contraction steps over dt pairs
                    for ht in range(HT):
                        # full 2KB bank: PSUM zero regions are bank-granular,
                        # so concurrent accumulation groups must not share one
                        ph_full = hpsum.tile([P, L1_CHUNK], f32, tag="ph")
                        ph = ph_full[:, :W]
                        for j in range(DT // 2):
                            nc.tensor.matmul(
                                ph[:, :],
                                w1v[:, ht, 2 * j:2 * j + 2, :],
                                ztv[:, 2 * j:2 * j + 2, c0:c0 + W],
                                start=(j == 0),
                                stop=(j == DT // 2 - 1),
                                perf_mode=DR,
                            )
                        # relu + downcast to e4m3 (values <= ~160 < 240 max),
                        # alternating DVE / ACT so neither engine bottlenecks
                        if has_b1:
                            nc.scalar.activation(
                                hv[:, ht, :], ph[:, :], Relu,
                                bias=b1_sb[:, s * HT + ht: s * HT + ht + 1],
                            )
                        elif ht % 2 == 0:
                            nc.vector.tensor_max(hv[:, ht, :], ph[:, :],
                                                 zero_sb[:, :W])
                        else:
                            nc.scalar.activation(hv[:, ht, :], ph[:, :], Relu)
                    # layer 2: psum[dt] = 512*u^T[dt] = (8*W2)^T (64*h^T),
                    # j-pairs outer (matches w2 load order + h production
                    # order), 4 live PSUM banks accumulate the dt tiles
                    u_sb = usb.tile([P, DT * W], bf16, tag="u")
                    pu = [upsum.tile([P, L1_CHUNK], f32, tag=f"pu{dt}", name=f"pu{dt}")[:, :W]
                          for dt in range(DT)]
                    for j in range(HT // 2):
                        for dt in range(DT):
                            nc.tensor.matmul(
                                pu[dt][:, :],
                                w2v[:, 2 * j:2 * j + 2, dt * P:(dt + 1) * P],
                                hv[:, 2 * j:2 * j + 2, :],
                                start=(j == 0),
                                stop=(j == HT // 2 - 1),
                                perf_mode=DR,
                            )
                    for dt in range(DT):
                        if eng % 2 == 0:
                            nc.vector.tensor_copy(u_sb[:, dt * W:(dt + 1) * W],
                                                  pu[dt][:, :])
                        else:
                            nc.scalar.copy(u_sb[:, dt * W:(dt + 1) * W],
                                           pu[dt][:, :])
                        eng += 1
                    # output store on the same Sync HWDGE ring (it queues
                    # after the input loads; avoids the 2.5us gpsimd SWDGE
                    # drain in the kernel tail); u_d is slot-major
                    # [p, slot:[dt, C]] so the single-chunk case is contiguous
                    if W == C:
                        nc.sync.dma_start(
                            u_d[:, DT * off: DT * (off + C)], u_sb[:, :])
                    else:
                        nc.sync.dma_start(
                            u_d[:, DT * off: DT * (off + C)].rearrange(
                                "p (d c) -> p d c", d=DT)[:, :, c0:c0 + W],
                            u_sb.rearrange("p (d c) -> p d c", d=DT),
                        )
    nc.compile()
    return nc


def _get_nc(Cs, has_b1):
    key = (tuple(int(c) for c in Cs), bool(has_b1))
    if key not in _nc_cache:
        _nc_cache[key] = _build_nc(key[0], key[1])
    return _nc_cache[key]


def kernel(inp, ln_g, ln_b, wg_group, wg_inner, W1, b1, W2, b2, gln_g, gln_b):
    global LAST_RESULTS
    import jax
    import jax.numpy as jnp
    import ml_dtypes

    inp = np.asarray(inp)
    in_dtype = inp.dtype
    fp8 = ml_dtypes.float8_e4m3  # TRN FP8_EXP4 (max 240), matches dt.float8e4

    # ---- 1. routing on host (bit-exact replica of the reference gates)
    gi, gsc, z, eis, escs = _routing(inp, ln_g, ln_b, wg_group, wg_inner)
    x = np.asarray(inp, np.float32).reshape(-1, D)

    # token lists per (g, e)
    tok_lists, scale_lists = {}, {}
    for g in range(G):
        in_g = (gi == g).any(axis=1)
        S_g = np.nonzero(in_g)[0]
        ei, esc = eis[g], escs[g]
        for e in range(E):
            sel = ei[S_g] == e           # [|S_g|, EK]
            has = sel.any(axis=1)
            toks = S_g[has]
            w = (esc[S_g] * sel).sum(axis=1)[has]
            tok_lists[(g, e)] = toks
            scale_lists[(g, e)] = w.astype(np.float32)

    # ---- 2. balanced assignment of the 32 pairs to (core, slot)
    pairs = [(g, e) for g in range(G) for e in range(E)]
    pairs.sort(key=lambda p: -len(tok_lists[p]))
    # rank r (descending capacity) -> slot position: small slot first (fast
    # PE start behind the DMA stream), smallest slot last (small final store)
    perm = [1, 2, 0, 3] if SLOTS == 4 else list(range(SLOTS))  # rank -> slot
    assign = {}           # (core, slot) -> (g, e)
    Cs = [0] * SLOTS
    for r in range(SLOTS):
        rank = pairs[r * NCORES:(r + 1) * NCORES]
        s = perm[r]
        Cs[s] = max(CAP_GRAN, _round_up(max(len(tok_lists[p]) for p in rank), CAP_GRAN))
        for c, p in enumerate(rank):
            assign[(c, s)] = p
    CT = int(sum(Cs))
    offs = np.concatenate([[0], np.cumsum(Cs)]).astype(int)

    # ---- 3. build per-core input maps (fp8, static scales)
    W1n = np.asarray(W1, np.float32)
    W2n = np.asarray(W2, np.float32)
    b1n = np.asarray(b1, np.float32)
    b2n = np.asarray(b2, np.float32)
    z8 = (z * SZ).astype(fp8)

    in_maps = []
    for c in range(NCORES):
        # partition-major device layouts (see _build_nc)
        zt_np = np.zeros((P, DT * CT), fp8)
        w1_np = np.empty((SLOTS, P, HT * DT * P), fp8)
        w2_np = np.empty((SLOTS, P, HT * D), fp8)
        b1_np = np.empty((P, SLOTS * HT), np.float32)
        b1_v = b1_np.reshape(P, SLOTS, HT)
        for s in range(SLOTS):
            g, e = assign[(c, s)]
            toks = tok_lists[(g, e)]
            n = len(toks)
            off = int(offs[s])
            C = int(Cs[s])
            # z^T slot region [p, dt, c]
            reg = zt_np[:, DT * off: DT * (off + C)].reshape(P, DT, C)
            reg[:, :, :n] = z8[toks].T.reshape(DT, P, n).transpose(1, 0, 2)
            # w1 [p, ht, dt, c] = 8*W1[dt*128+p, ht*128+c]
            w1_np[s] = (
                (W1n[g, e] * SW1).astype(fp8)
                .reshape(DT, P, HT, P).transpose(1, 2, 0, 3).reshape(P, HT * DT * P)
            )
            # w2 [p, ht, d] = 8*W2[ht*128+p, d]
            w2_np[s] = (
                (W2n[g, e] * SW2).astype(fp8)
                .reshape(HT, P, D).transpose(1, 0, 2).reshape(P, HT * D)
            )
            # bias lands in psum scale (SZ*SW1)
            b1_v[:, s, :] = (b1n[g, e] * (SZ * SW1)).reshape(HT, P).T
        in_maps.append({"zt": zt_np, "w1": w1_np, "w2": w2_np, "b1": b1_np})

    # ---- 4. compile + run on the 8 NeuronCores
    _ensure_ntff_hook()
    from concourse.bass_utils import run_bass_kernel_spmd

    nc = _get_nc(Cs, has_b1=bool(np.any(b1n)))
    res = run_bass_kernel_spmd(
        nc, in_maps, core_ids=list(range(NCORES)),
        trace=bool(int(os.environ.get("KERNEL_TRACE", "0"))),
    )
    LAST_RESULTS = res

    # ---- 5. host combine
    moe = np.zeros((G, N, D), np.float32)
    for c in range(NCORES):
        uc = np.asarray(res.results[c]["u"], np.float32)
        for s in range(SLOTS):
            g, e = assign[(c, s)]
            toks = tok_lists[(g, e)]
            n = len(toks)
            off = int(offs[s])
            C = int(Cs[s])
            # u slot region [p, dt, c] -> [c, dt*128+p] = 512*u[token, d]
            u = (
                uc[:, DT * off: DT * (off + C)]
                .reshape(P, DT, C).transpose(1, 0, 2).reshape(D, C).T
            )
            w = scale_lists[(g, e)]
            contrib = u[:n] * (w / USCALE)[:, None] + w[:, None] * b2n[g, e][None, :]
            np.add.at(moe[g], toks, contrib)

    cpu = jax.devices("cpu")[0]
    with jax.default_device(cpu):
        zj = jnp.asarray(z)
        gi_j = jnp.asarray(gi)
        gsc_j = jnp.asarray(gsc)
        gw_dense = jnp.sum(
            jax.nn.one_hot(gi_j, G, dtype=jnp.float32) * gsc_j[..., None], axis=-2
        )  # [N, G]
        out = jnp.zeros((N, D), jnp.float32)
        gg = jnp.asarray(np.asarray(gln_g, np.float32))
        gb = jnp.asarray(np.asarray(gln_b, np.float32))
        for g in range(G):
            t = zj + jnp.asarray(moe[g])
            m = jnp.mean(t, axis=-1, keepdims=True)
            tc_ = t - m
            v = jnp.mean(tc_ * tc_, axis=-1, keepdims=True)
            y = tc_ * jax.lax.rsqrt(v + EPS) * gg[g] + gb[g]
            out = out + gw_dense[:, g:g + 1] * y
        result = np.asarray(out).reshape(B, T, D) + np.asarray(inp, np.float32)

    return result.astype(in_dtype)


# revision 25
# speedup vs baseline: 1.8107x; 1.0199x over previous
"""Trainium2 kernel for nn_CustomizedMoGPositionwiseFF (moe_routing).

Strategy (expert-parallel, per the sharding hint):
  - 32 (group, expert) FFN pairs are sharded across 8 NeuronCores (4 each).
  - Routing (group top-2 gate + per-group inner top-2 gate) is computed on
    host at call time; tokens are dispatched (gathered) per expert into the
    per-core shards -- data-dependent sharding, compiled into the NEFF.
  - Each core runs both FFN matmuls + relu for its 4 experts over the tokens
    routed to them, reading each expert weight exactly once (memory regime).
  - Weights/activations are shipped as fp8 e4m3 (TRN FP8_EXP4, max 240) with
    static scales (z*8, W1*8 -> psum = 64*h; relu+cast to e4m3; W2*8 ->
    psum = 512*u, stored bf16, divided by 512 on host).  Matmuls run in
    DoubleRow perf mode: 256-deep contraction, 2 fp8 MACs/cell/cycle.
  - DMA loads are issued in consumption order in ~0.25MB chunks alternating
    between the two HWDGE rings so the PE starts ~1us in and never starves.
  - Host applies the cheap O(N*D) combine: iw/b2 scaling, scatter-add of the
    two expert contributions per (token, group), per-group post-layernorm,
    group top-2 mixture, and the outer residual.
"""

import os
import numpy as np

# Model dims (hardcoded per the contract; match the reference problem)
B, T, D, H = 2, 1024, 512, 2048
G, E, GK, EK = 4, 8, 2, 2
EPS = 1e-5
N = B * T
P = 128
DT = D // P    # 4 d-tiles
HT = H // P    # 16 h-tiles
NCORES = 8
SLOTS = (G * E) // NCORES  # 4 experts per core
CAP_GRAN = 16              # capacity granularity (tokens)
L1_CHUNK = 512             # moving-dim chunk for both layers (one PSUM bank)

# fp8 static scales: psum1 = SZ*SW1*h ; h stored as e4m3 at scale SH=SZ*SW1;
# psum2 = SH*SW2*u (bf16 out, divided on host).
SZ = 8.0
SW1 = 8.0
SW2 = 8.0
USCALE = SZ * SW1 * SW2    # 512

_nc_cache = {}
LAST_RESULTS = None       # test harness can inspect (BassKernelResults)


def _ensure_ntff_hook():
    """Register antenv.axon_hooks with the ctypes NTFF profile hook if the
    container's antenv package lacks it (mirrors trn_agent_boot.trn_boot).
    Makes trace=True work; degrades to hook=None when the .so is absent."""
    try:
        from antenv.axon_hooks import get_axon_ntff_profile_hook  # noqa: F401
        return
    except ImportError:
        pass
    import sys
    import types
    import contextlib
    import ctypes

    mod = types.ModuleType("antenv.axon_hooks")
    _state = {"hook": None}

    def set_axon_ntff_profile_hook(h):
        _state["hook"] = h

    def get_axon_ntff_profile_hook():
        return _state["hook"]

    mod.set_axon_ntff_profile_hook = set_axon_ntff_profile_hook
    mod.get_axon_ntff_profile_hook = get_axon_ntff_profile_hook

    so_path = "/opt/axon/libaxon_pjrt.so"
    hook = None
    if os.path.exists(so_path):
        try:
            lib = ctypes.CDLL(so_path)
            if hasattr(lib, "axon_start_nrt_profile"):
                lib.axon_start_nrt_profile.argtypes = [
                    ctypes.POINTER(ctypes.c_int64), ctypes.c_size_t]
                lib.axon_start_nrt_profile.restype = ctypes.c_int64
                lib.axon_stop_nrt_profile.argtypes = [ctypes.c_char_p]
                lib.axon_stop_nrt_profile.restype = ctypes.c_int64

                @contextlib.contextmanager
                def _hook(output_dir, device_ids):
                    import jax
                    jax.devices()
                    if device_ids:
                        ids = (ctypes.c_int64 * len(device_ids))(*device_ids)
                        rc = lib.axon_start_nrt_profile(ids, len(device_ids))
                    else:
                        rc = lib.axon_start_nrt_profile(None, 0)
                    if rc != 0:
                        raise RuntimeError(f"axon_start_nrt_profile rc={rc}")
                    try:
                        yield
                    finally:
                        n = lib.axon_stop_nrt_profile(str(output_dir).encode())
                        print(f"ntff profile: {n} file(s) -> {output_dir}")

                hook = _hook
        except Exception:
            hook = None
    _state["hook"] = hook
    import antenv
    sys.modules["antenv.axon_hooks"] = mod
    antenv.axon_hooks = mod


def _round_up(x, m):
    return ((x + m - 1) // m) * m


def _routing(inp, ln_g, ln_b, wg_group, wg_inner):
    """Replicate the reference gating bit-for-bit on jax-cpu.

    Returns gi [N,GK] group ids, gsc [N,GK] group softmax, z [N,D] f32,
    eis/escs: per-group inner top-k ids/softmax ([N,EK] each).
    """
    import jax
    import jax.numpy as jnp

    cpu = jax.devices("cpu")[0]
    with jax.default_device(cpu):
        x = jnp.asarray(np.asarray(inp, np.float32)).reshape(-1, D)
        gl = x @ jnp.asarray(np.asarray(wg_group, np.float32))
        gv, gi = jax.lax.top_k(gl, GK)
        gsc = jax.nn.softmax(gv, axis=-1)
        m = jnp.mean(x, axis=-1, keepdims=True)
        xc = x - m
        v = jnp.mean(xc * xc, axis=-1, keepdims=True)
        z = xc * jax.lax.rsqrt(v + EPS) * jnp.asarray(np.asarray(ln_g, np.float32)) \
            + jnp.asarray(np.asarray(ln_b, np.float32))
        wgi = jnp.asarray(np.asarray(wg_inner, np.float32))
        eis, escs = [], []
        for g in range(G):
            l = z @ wgi[g]
            ev, ei = jax.lax.top_k(l, EK)
            esc = jax.nn.softmax(ev, axis=-1)
            eis.append(np.asarray(ei))
            escs.append(np.asarray(esc))
    return np.asarray(gi), np.asarray(gsc), np.asarray(z), eis, escs


def _build_nc(Cs, has_b1=False):
    """Build the SPMD Bass program for per-slot capacities Cs (uniform across cores)."""
    import concourse.bass as bass
    import concourse.bacc as bacc
    import concourse.tile as tile
    from concourse import mybir

    f32 = mybir.dt.float32
    bf16 = mybir.dt.bfloat16
    fp8 = mybir.dt.float8e4
    DR = mybir.MatmulPerfMode.DoubleRow
    Relu = mybir.ActivationFunctionType.Relu

    CT = int(sum(Cs))
    offs = np.concatenate([[0], np.cumsum(Cs)]).astype(int)

    nc = bacc.Bacc("TRN2", target_bir_lowering=False)
    # all DRAM layouts are partition-major [128, free] so every DMA is 128
    # contiguous lines (max-size descriptors, cheap HWDGE issue)
    zt_d = nc.declare_dram_parameter("zt", [P, DT * CT], fp8, isOutput=False)
    w1_d = nc.declare_dram_parameter("w1", [SLOTS, P, HT * DT * P], fp8, isOutput=False)
    w2_d = nc.declare_dram_parameter("w2", [SLOTS, P, HT * D], fp8, isOutput=False)
    b1_d = nc.declare_dram_parameter("b1", [P, SLOTS * HT], f32, isOutput=False)
    u_d = nc.declare_dram_parameter("u", [P, DT * CT], bf16, isOutput=True)

    W1CH = 2   # w1 load chunks per slot
    W2CH = 2   # w2 load chunks per slot

    WARMUP = 12  # dummy PE matmuls bridging the preamble -> first-data gap

    with tile.TileContext(nc) as tc:
        with tc.tile_pool(name="consts", bufs=1) as consts, \
             tc.tile_pool(name="hpool", bufs=2) as hpool, \
             tc.tile_pool(name="hpsum", bufs=3, space="PSUM") as hpsum, \
             tc.tile_pool(name="upsum", bufs=1, space="PSUM") as upsum, \
             tc.tile_pool(name="wpsum", bufs=1, space="PSUM") as wpsum, \
             tc.tile_pool(name="usb", bufs=2) as usb:

            zt_sb = consts.tile([P, DT * CT], fp8, tag="zt")
            b1_sb = consts.tile([P, SLOTS * HT], f32, tag="b1")
            zero_sb = consts.tile([P, L1_CHUNK], f32, tag="zero")
            zf8_sb = consts.tile([P, L1_CHUNK], fp8, tag="zf8")
            # zf8 first: it gates the HAM-warmup matmuls
            nc.vector.memset(zf8_sb[:, :], 0.0)
            nc.vector.memset(zero_sb[:, :], 0.0)
            w1_sb, w2_sb = [], []
            for s in range(SLOTS):
                w1_sb.append(consts.tile([P, HT * DT * P], fp8, tag=f"w1_{s}", name=f"w1s_{s}"))
                w2_sb.append(consts.tile([P, HT * D], fp8, tag=f"w2_{s}", name=f"w2s_{s}"))

            # ---- streaming loads, issued in exact consumption order, ALL on
            # the Sync engine's HWDGE ring.  Sync runs no compute, so the
            # lane-reuse semaphore waits that pace DMA issue to transfer
            # completion can't block anything (issuing from Scalar serialized
            # the ACT relus behind 25us of paced DMA issues).  One logical
            # queue also makes delivery order == issue order (FIFO per ring).
            def _load(dst, src):
                nc.sync.dma_start(dst, src)

            if has_b1:
                _load(b1_sb[:, :], b1_d[:, :])
            for s in range(SLOTS):
                C = int(Cs[s])
                off = int(offs[s])
                _load(zt_sb[:, DT * off: DT * (off + C)],
                      zt_d[:, DT * off: DT * (off + C)])
                # slot 0's first w1 chunk is small so the first real matmul
                # can start as early as possible
                hts = [2, 6, 8] if s == 0 else [8, 8]
                h0 = 0
                for nh in hts:
                    _load(w1_sb[s][:, h0 * DT * P:(h0 + nh) * DT * P],
                          w1_d[s][:, h0 * DT * P:(h0 + nh) * DT * P])
                    h0 += nh
                step2 = (HT // W2CH) * D
                for i in range(W2CH):
                    _load(w2_sb[s][:, i * step2:(i + 1) * step2],
                          w2_d[s][:, i * step2:(i + 1) * step2])

            # ---- HAM warmup: the tile-framework preamble (barriers + engine
            # table loads) plus first-chunk DMA latency leaves the PE idle
            # for ~10us, which re-throttles its clock to 1.2 GHz for the
            # first ~3.4us of real work.  Bridge the gap with dummy
            # DoubleRow matmuls on an all-zero fp8 tile: they depend only on
            # the memset, keep the PE busy from ~3.5us on, and finish right
            # as the first weight chunk lands.
            zf8v = zf8_sb.rearrange("p (two c) -> p two c", two=2)
            wps = wpsum.tile([P, L1_CHUNK], f32, tag="wps")
            for k in range(WARMUP):
                nc.tensor.matmul(
                    wps[:, :L1_CHUNK // 2], zf8v[:, :, :P], zf8v[:, :, :],
                    start=True, stop=True, perf_mode=DR)

            # ---- compute
            for s in range(SLOTS):
                C = int(Cs[s])
                off = int(offs[s])
                # [P, DT, C] view of this slot's z^T shard
                ztv = zt_sb[:, DT * off: DT * (off + C)].rearrange(
                    "p (dt c) -> p dt c", dt=DT)
                w1v = w1_sb[s].rearrange("p (ht dt c) -> p ht dt c", ht=HT, dt=DT)
                w2v = w2_sb[s].rearrange("p (ht d) -> p ht d", ht=HT)
                eng = 0
                for c0 in range(0, C, L1_CHUNK):
                    W = min(L1_CHUNK, C - c0)
                    h_sb = hpool.tile([P, HT * W], fp8, tag="h")
                    hv = h_sb.rearrange("p (ht c) -> p ht c", ht=HT)
                    # layer 1: psum[ht] = 64*h^T[ht] = (8*W1)^T (8*z^T),
                    # DoubleRow: two 256-deep contraction steps over dt pairs
                    for ht in range(HT):
                        # full 2KB bank: PSUM zero regions are bank-granular,
                        # so concurrent accumulation groups must not share one
                        ph_full = hpsum.tile([P, L1_CHUNK], f32, tag="ph")
                        ph = ph_full[:, :W]
                        for j in range(DT // 2):
                            nc.tensor.matmul(
                                ph[:, :],
                                w1v[:, ht, 2 * j:2 * j + 2, :],
                                ztv[:, 2 * j:2 * j + 2, c0:c0 + W],
                                start=(j == 0),
                                stop=(j == DT // 2 - 1),
                                perf_mode=DR,
                            )
                        # relu + downcast to e4m3 (values <= ~160 < 240 max),
                        # alternating DVE / ACT so neither engine bottlenecks
                        if has_b1:
                            nc.scalar.activation(
                                hv[:, ht, :], ph[:, :], Relu,
                                bias=b1_sb[:, s * HT + ht: s * HT + ht + 1],
                            )
                        elif ht % 2 == 0:
                            nc.vector.tensor_max(hv[:, ht, :], ph[:, :],
                                                 zero_sb[:, :W])
                        else:
                            nc.scalar.activation(hv[:, ht, :], ph[:, :], Relu)
                    # layer 2: psum[dt] = 512*u^T[dt] = (8*W2)^T (64*h^T),
                    # j-pairs outer (matches w2 load order + h production
                    # order), 4 live PSUM banks accumulate the dt tiles
                    u_sb = usb.tile([P, DT * W], bf16, tag="u")
                    pu = [upsum.tile([P, L1_CHUNK], f32, tag=f"pu{dt}", name=f"pu{dt}")[:, :W]
                          for dt in range(DT)]
                    last = (s == SLOTS - 1) and (c0 + W >= C) and (W == C)
                    udv = u_d[:, DT * off: DT * (off + C)]

                    def _ucopy(dt):
                        nonlocal eng
                        if eng % 2 == 0:
                            nc.vector.tensor_copy(u_sb[:, dt * W:(dt + 1) * W],
                                                  pu[dt][:, :])
                        else:
                            nc.scalar.copy(u_sb[:, dt * W:(dt + 1) * W],
                                           pu[dt][:, :])
                        eng += 1

                    if last:
                        # final slot: dt-outer with per-dt copy + store so the
                        # tail store is small and overlaps the last matmuls
                        for dt in range(DT):
                            for j in range(HT // 2):
                                nc.tensor.matmul(
                                    pu[dt][:, :],
                                    w2v[:, 2 * j:2 * j + 2, dt * P:(dt + 1) * P],
                                    hv[:, 2 * j:2 * j + 2, :],
                                    start=(j == 0),
                                    stop=(j == HT // 2 - 1),
                                    perf_mode=DR,
                                )
                            _ucopy(dt)
                            nc.sync.dma_start(udv[:, dt * W:(dt + 1) * W],
                                              u_sb[:, dt * W:(dt + 1) * W])
                    else:
                        for j in range(HT // 2):
                            for dt in range(DT):
                                nc.tensor.matmul(
                                    pu[dt][:, :],
                                    w2v[:, 2 * j:2 * j + 2, dt * P:(dt + 1) * P],
                                    hv[:, 2 * j:2 * j + 2, :],
                                    start=(j == 0),
                                    stop=(j == HT // 2 - 1),
                                    perf_mode=DR,
                                )
                        for dt in range(DT):
                            _ucopy(dt)
                        # output store on the Sync HWDGE ring (it queues after
                        # the input loads; avoids the gpsimd SWDGE drain in
                        # the kernel tail); u_d is slot-major [p, slot:[dt, C]]
                        # so the single-chunk case is contiguous
                        if W == C:
                            nc.sync.dma_start(udv[:, :], u_sb[:, :])
                        else:
                            nc.sync.dma_start(
                                udv.rearrange("p (d c) -> p d c", d=DT)[:, :, c0:c0 + W],
                                u_sb.rearrange("p (d c) -> p d c", d=DT),
                            )
    nc.compile()
    return nc


def _get_nc(Cs, has_b1):
    key = (tuple(int(c) for c in Cs), bool(has_b1))
    if key not in _nc_cache:
        _nc_cache[key] = _build_nc(key[0], key[1])
    return _nc_cache[key]


def kernel(inp, ln_g, ln_b, wg_group, wg_inner, W1, b1, W2, b2, gln_g, gln_b):
    global LAST_RESULTS
    import jax
    import jax.numpy as jnp
    import ml_dtypes

    inp = np.asarray(inp)
    in_dtype = inp.dtype
    fp8 = ml_dtypes.float8_e4m3  # TRN FP8_EXP4 (max 240), matches dt.float8e4

    # ---- 1. routing on host (bit-exact replica of the reference gates)
    gi, gsc, z, eis, escs = _routing(inp, ln_g, ln_b, wg_group, wg_inner)
    x = np.asarray(inp, np.float32).reshape(-1, D)

    # token lists per (g, e)
    tok_lists, scale_lists = {}, {}
    for g in range(G):
        in_g = (gi == g).any(axis=1)
        S_g = np.nonzero(in_g)[0]
        ei, esc = eis[g], escs[g]
        for e in range(E):
            sel = ei[S_g] == e           # [|S_g|, EK]
            has = sel.any(axis=1)
            toks = S_g[has]
            w = (esc[S_g] * sel).sum(axis=1)[has]
            tok_lists[(g, e)] = toks
            scale_lists[(g, e)] = w.astype(np.float32)

    # ---- 2. balanced assignment of the 32 pairs to (core, slot)
    pairs = [(g, e) for g in range(G) for e in range(E)]
    pairs.sort(key=lambda p: -len(tok_lists[p]))
    # rank r (descending capacity) -> slot position: small slot first (fast
    # PE start behind the DMA stream), smallest slot last (small final store)
    perm = [1, 2, 0, 3] if SLOTS == 4 else list(range(SLOTS))  # rank -> slot
    assign = {}           # (core, slot) -> (g, e)
    Cs = [0] * SLOTS
    for r in range(SLOTS):
        rank = pairs[r * NCORES:(r + 1) * NCORES]
        s = perm[r]
        Cs[s] = max(CAP_GRAN, _round_up(max(len(tok_lists[p]) for p in rank), CAP_GRAN))
        for c, p in enumerate(rank):
            assign[(c, s)] = p
    CT = int(sum(Cs))
    offs = np.concatenate([[0], np.cumsum(Cs)]).astype(int)

    # ---- 3. build per-core input maps (fp8, static scales)
    W1n = np.asarray(W1, np.float32)
    W2n = np.asarray(W2, np.float32)
    b1n = np.asarray(b1, np.float32)
    b2n = np.asarray(b2, np.float32)
    z8 = (z * SZ).astype(fp8)

    in_maps = []
    for c in range(NCORES):
        # partition-major device layouts (see _build_nc)
        zt_np = np.zeros((P, DT * CT), fp8)
        w1_np = np.empty((SLOTS, P, HT * DT * P), fp8)
        w2_np = np.empty((SLOTS, P, HT * D), fp8)
        b1_np = np.empty((P, SLOTS * HT), np.float32)
        b1_v = b1_np.reshape(P, SLOTS, HT)
        for s in range(SLOTS):
            g, e = assign[(c, s)]
            toks = tok_lists[(g, e)]
            n = len(toks)
            off = int(offs[s])
            C = int(Cs[s])
            # z^T slot region [p, dt, c]
            reg = zt_np[:, DT * off: DT * (off + C)].reshape(P, DT, C)
            reg[:, :, :n] = z8[toks].T.reshape(DT, P, n).transpose(1, 0, 2)
            # w1 [p, ht, dt, c] = 8*W1[dt*128+p, ht*128+c]
            w1_np[s] = (
                (W1n[g, e] * SW1).astype(fp8)
                .reshape(DT, P, HT, P).transpose(1, 2, 0, 3).reshape(P, HT * DT * P)
            )
            # w2 [p, ht, d] = 8*W2[ht*128+p, d]
            w2_np[s] = (
                (W2n[g, e] * SW2).astype(fp8)
                .reshape(HT, P, D).transpose(1, 0, 2).reshape(P, HT * D)
            )
            # bias lands in psum scale (SZ*SW1)
            b1_v[:, s, :] = (b1n[g, e] * (SZ * SW1)).reshape(HT, P).T
        in_maps.append({"zt": zt_np, "w1": w1_np, "w2": w2_np, "b1": b1_np})

    # ---- 4. compile + run on the 8 NeuronCores
    _ensure_ntff_hook()
    from concourse.bass_utils import run_bass_kernel_spmd

    nc = _get_nc(Cs, has_b1=bool(np.any(b1n)))
    res = run_bass_kernel_spmd(
        nc, in_maps, core_ids=list(range(NCORES)),
        trace=bool(int(os.environ.get("KERNEL_TRACE", "0"))),
    )
    LAST_RESULTS = res

    # ---- 5. host combine
    moe = np.zeros((G, N, D), np.float32)
    for c in range(NCORES):
        uc = np.asarray(res.results[c]["u"], np.float32)
        for s in range(SLOTS):
            g, e = assign[(c, s)]
            toks = tok_lists[(g, e)]
            n = len(toks)
            off = int(offs[s])
            C = int(Cs[s])
            # u slot region [p, dt, c] -> [c, dt*128+p] = 512*u[token, d]
            u = (
                uc[:, DT * off: DT * (off + C)]
                .reshape(P, DT, C).transpose(1, 0, 2).reshape(D, C).T
            )
            w = scale_lists[(g, e)]
            contrib = u[:n] * (w / USCALE)[:, None] + w[:, None] * b2n[g, e][None, :]
            np.add.at(moe[g], toks, contrib)

    cpu = jax.devices("cpu")[0]
    with jax.default_device(cpu):
        zj = jnp.asarray(z)
        gi_j = jnp.asarray(gi)
        gsc_j = jnp.asarray(gsc)
        gw_dense = jnp.sum(
            jax.nn.one_hot(gi_j, G, dtype=jnp.float32) * gsc_j[..., None], axis=-2
        )  # [N, G]
        out = jnp.zeros((N, D), jnp.float32)
        gg = jnp.asarray(np.asarray(gln_g, np.float32))
        gb = jnp.asarray(np.asarray(gln_b, np.float32))
        for g in range(G):
            t = zj + jnp.asarray(moe[g])
            m = jnp.mean(t, axis=-1, keepdims=True)
            tc_ = t - m
            v = jnp.mean(tc_ * tc_, axis=-1, keepdims=True)
            y = tc_ * jax.lax.rsqrt(v + EPS) * gg[g] + gb[g]
            out = out + gw_dense[:, g:g + 1] * y
        result = np.asarray(out).reshape(B, T, D) + np.asarray(inp, np.float32)

    return result.astype(in_dtype)
